# revision 54
# baseline (speedup 1.0000x reference)
"""AttentionWithFastKANTransform Trainium2 kernel (8 NeuronCores, SPMD).

v2 design:
  phase 1 (row-sharded, R=512 rows/core): FastKAN lq/lk/lv/lg with fp8
    DoubleRow spline matmuls (4x fewer PE cycles). RBF basis built by a
    bf16 multiply chain on DVE (b_{j+1} = b_j * rc_j, rc_{j+1} = rc_j*e^-2)
    seeded by two ACT exps, then converted to fp8 tiles for the matmuls.
  AllToAlls (fp8): wq/wk -> [32,2ko,L] per head; wv locally PE-transposed
    to [k,d] before the a2a; sigmoid gate bf16.
  phase 2 (head-sharded): S = wk^T wq fp8 DoubleRow (k-partitioned, 32x2
    contraction), exp on ACT -> fp8 A pair-tiles, att@V fp8 DoubleRow over
    k-tile pairs with a ones-column for softmax denominators.
  Gated output a2a'd back in two halves (bf16) so phase 3 overlaps phase 2.
  phase 3: FastKAN lo with bf16 spline (fp8 too lossy for the final layer),
    split in two row-halves for overlap.
"""

import os
import numpy as np
import ml_dtypes

import concourse.bass as bass
import concourse.bacc as bacc
import concourse.tile as tile
import concourse.mybir as mybir
from concourse.bass_utils import run_bass_kernel_spmd
from concourse.masks import make_identity

AF = mybir.ActivationFunctionType
OP = mybir.AluOpType
F32 = mybir.dt.float32
BF16 = mybir.dt.bfloat16
F8 = mybir.dt.float8e4
F8NP = ml_dtypes.float8_e4m3fn
BFNP = ml_dtypes.bfloat16

NCORES = 8
B, L, IN, OUT, H, D, G = 2, 2048, 512, 512, 8, 64, 8
R = (B * L) // NCORES          # 512 rows per core
NC_IN = IN // 128              # 4 input-dim chunks
NKT = L // 128                 # 16 k-tiles per batch
STEP = 4.0 / (G - 1)
EPS = 1e-5
QC = 512                       # phase-2 q-chunk
NQC = L // QC                  # 4
F8_LAYERS = ("lq", "lk", "lv", "lg")
RHO = float(np.exp(-2.0))

_cache = {}


def _patch_act_tables():
    """Prefer the ln+exp combined table so Ln/Exp don't ping-pong loads."""
    if _cache.get("act_patched"):
        return
    import concourse.bacc as _bacc
    import concourse.hw_specs as _hw
    orig = _hw.get_activation_tables

    def reordered(arch):
        t = dict(orig(arch))
        order = ["natural_log_exp_and_others", "silu_and_others"]
        out = {k: t[k] for k in order if k in t}
        out.update({k: v for k, v in t.items() if k not in out})
        return out

    _bacc.get_activation_tables = reordered
    _cache["act_patched"] = True


def _build_program(ws):
    """ws: dict layer -> fp8 weight scale (host-derived, baked as consts)."""
    _patch_act_tables()
    nc = bacc.Bacc("TRN2", target_bir_lowering=False, debug=False,
                   num_devices=NCORES)
    io = {}
    io["xT3"] = nc.dram_tensor("xT3", [3, IN, R], BF16, kind="ExternalInput").ap()
    for l in F8_LAYERS:
        io[l + "_sw8"] = nc.dram_tensor(l + "_sw8", [4, 128, 16, 2, 128], F8,
                                        kind="ExternalInput").ap()
    io["lo_swp"] = nc.dram_tensor("lo_swp", [G, NC_IN, 128, OUT], BF16,
                                  kind="ExternalInput").ap()
    for l in F8_LAYERS + ("lo",):
        io[l + "_bwp"] = nc.dram_tensor(l + "_bwp", [NC_IN, 128, OUT], BF16,
                                        kind="ExternalInput").ap()
    io["outT"] = nc.dram_tensor("outT", [2, 128, 2, R], BF16,
                                kind="ExternalOutput").ap()

    rg = [list(range(NCORES))]
    nocc = bool(int(os.environ.get("KERNEL_NOCC", "0")))
    stop = int(os.environ.get("KERNEL_STOP", "9"))

    with tile.TileContext(nc) as tc:
        with tc.tile_pool(name="dram1", bufs=1, space="DRAM") as dram1, \
             tc.tile_pool(name="sb", bufs=2) as sb, \
             tc.tile_pool(name="sb3", bufs=3) as sb3, \
             tc.tile_pool(name="ub", bufs=3) as ubp, \
             tc.tile_pool(name="ubo", bufs=8) as ubop, \
             tc.tile_pool(name="wt", bufs=2) as wtp, \
             tc.tile_pool(name="consts", bufs=1) as cpool, \
             tc.tile_pool(name="ps_mm", bufs=2, space="PSUM") as ps_mm, \
             tc.tile_pool(name="ps_s", bufs=2, space="PSUM") as ps_s:

            # ---------------- collective buffers
            a_qk_i = dram1.tile([NCORES, 2, D, R], F8, tag="aqk_i")
            a_qk_o = dram1.tile([NCORES, 2, D, R], F8, tag="aqk_o")
            a_sg_i = dram1.tile([NCORES, D, R], BF16, tag="asg_i")
            a_sg_o = dram1.tile([NCORES, D, R], BF16, tag="asg_o")
            a_wv_i = dram1.tile([NCORES, R, D], F8, tag="awv_i")
            a_wv_o = dram1.tile([NCORES, R, D], F8, tag="awv_o")
            a_oA_i = dram1.tile([NCORES, D, R // 2], BF16, tag="aoA_i")
            a_oA_o = dram1.tile([NCORES, D, R // 2], BF16, tag="aoA_o")
            a_oB_i = dram1.tile([NCORES, D, R // 2], BF16, tag="aoB_i")
            a_oB_o = dram1.tile([NCORES, D, R // 2], BF16, tag="aoB_o")

            def a2a(i, o):
                if nocc:
                    nc.sync.dma_start(o, i)
                else:
                    nc.gpsimd.collective_compute(
                        "AllToAll", OP.bypass, replica_groups=rg,
                        ins=[i.opt()], outs=[o.opt()])

            # ---------------- consts
            ones_col = cpool.tile([128, 1], BF16, tag="ones_col")
            nc.vector.memset(ones_col, 1.0 / IN)
            ones_row = cpool.tile([128, 128], BF16, tag="ones_row")
            nc.vector.memset(ones_row, 1.0)
            ones_rowf = cpool.tile([128, 128], F32, tag="ones_rowf")
            nc.vector.memset(ones_rowf, 1.0)
            ident8 = cpool.tile([128, 128], F8, tag="ident8")
            make_identity(nc, ident8)
            rho1 = cpool.tile([128, 1, R], BF16, tag="rho")
            nc.vector.memset(rho1, RHO)
            bm25 = cpool.tile([128, 1], F32, tag="bm25")
            nc.vector.memset(bm25, 3.5 - 6.0)
            bm35 = cpool.tile([128, 1], F32, tag="bm35")
            nc.vector.memset(bm35, 3.5 - 7.0)
            bm05 = cpool.tile([128, 1], F32, tag="bm05")
            nc.vector.memset(bm05, 3.5 - 4.0)
            bm20 = cpool.tile([128, 1], F32, tag="bm20")
            nc.vector.memset(bm20, 6.0 - 8.0)
            epst = cpool.tile([128, 1], F32, tag="eps")
            nc.vector.memset(epst, EPS)
            b35 = cpool.tile([128, 1], F32, tag="b35")
            nc.vector.memset(b35, 3.5)
            b60 = cpool.tile([128, 1], F32, tag="b60")
            nc.vector.memset(b60, 6.0)

            # PE warmup: keep the array busy during input DMA so the
            # first real matmuls run at full clock
            warm_ps = ps_s.tile([128, 2, R], F32, tag="S", name="warm")
            for w in range(10):
                nc.tensor.matmul(warm_ps[:, 0, 0:128], lhsT=ones_row,
                                 rhs=ones_row, start=True, stop=True,
                                 skip_group_check=True)

            # ---------------- x loads + batched silus (one table switch)
            def load_x(idx):
                x = sb3.tile([128, NC_IN, R], BF16, tag="x", name=f"x{idx}")
                nc.sync.dma_start(
                    x, io["xT3"][idx].rearrange("(c p) r -> p c r", p=128))
                return x

            xk, xq, xv = load_x(1), load_x(0), load_x(2)
            silus = {}
            for nm, x in (("k", xk), ("q", xq), ("v", xv)):
                s = sb3.tile([128, NC_IN, R], BF16, tag="silu", name=f"silu_{nm}")
                nc.scalar.activation(s, x, AF.Silu)
                silus[nm] = s

            # ---------------- batched LN stats (k,q,v in one Ln/Exp pair)
            def stats_batch(xs, cols=None):
                """xs: list of (x_sb, xsq_writer) tensors; returns list of
                (s_ap, t_ap) [1, n] access patterns per tensor."""
                lo_, hi_ = cols or (0, R)
                n = hi_ - lo_
                csl = slice(lo_, hi_)
                nt = len(xs)
                stat = ps_s.tile([97, 2, R], F32, tag="S", name="statb")
                for t, x_sb in enumerate(xs):
                    xsq = ubp.tile([128, NC_IN, R], BF16, tag="u",
                                   name=f"xsqb{t}")
                    for c in range(NC_IN):
                        nc.tensor.matmul(stat[32 * t:32 * t + 1, 0, csl],
                                         lhsT=ones_col, rhs=x_sb[:, c, csl],
                                         start=(c == 0), stop=(c == NC_IN - 1))
                    nc.vector.tensor_mul(xsq[:, :, csl], x_sb[:, :, csl],
                                         x_sb[:, :, csl])
                    for c in range(NC_IN):
                        nc.tensor.matmul(stat[32 * t:32 * t + 1, 1, csl],
                                         lhsT=ones_col, rhs=xsq[:, c, csl],
                                         start=(c == 0), stop=(c == NC_IN - 1))
                # stat rows now hold mu and E[x^2] directly (1/IN folded)
                sm = sb.tile([97, 3, R], F32, tag="stsm", bufs=1, name="smb")
                nc.gpsimd.memset(sm, 1.0)
                for t in range(nt):
                    p = slice(32 * t, 32 * t + 1)
                    var = sm[p, 1, csl]
                    # mumu = mu^2 (one PSUM operand is allowed)
                    nc.vector.scalar_tensor_tensor(sm[p, 2, csl],
                                                   stat[p, 0, csl], 1.0,
                                                   stat[p, 0, csl],
                                                   OP.mult, OP.mult)
                    nc.vector.tensor_sub(var, stat[p, 1, csl],
                                         sm[p, 2, csl])
                # one Ln + one Exp over all tensors (spread on partitions)
                nc.scalar.activation(sm[:, 2, csl], sm[:, 1, csl],
                                     AF.Ln, bias=epst[0:97])
                nc.scalar.activation(sm[:, 1, csl], sm[:, 2, csl],
                                     AF.Exp, scale=-0.5)
                for t in range(nt):
                    p = slice(32 * t, 32 * t + 1)
                    nc.vector.scalar_tensor_tensor(sm[p, 2, csl],
                                                   stat[p, 0, csl], -1.0,
                                                   sm[p, 1, csl],
                                                   OP.mult, OP.mult)
                return [(sm[32 * t:32 * t + 1, 1, :],
                         sm[32 * t:32 * t + 1, 2, :]) for t in range(nt)]

            # ---------------- prep stage A: xn (+ phase-3 silu)
            def prep_a(x_sb, nm, cols=None, silu_exp=False, stats=None):
                lo_, hi_ = cols or (0, R)
                n = hi_ - lo_
                csl = slice(lo_, hi_)

                if stats is None:
                    stats = stats_batch([x_sb], cols=cols)[0]
                s_ap, t_ap = stats
                # broadcast via PE: [1,n] -> [128,n] (two mms, one per bank)
                bp = s_ap.base_partition()
                orow = ones_rowf[bp:bp + 1, :].bitcast(mybir.dt.float32r)
                stb_ps = ps_s.tile([128, 2, R], F32, tag="S", name=f"stb_{nm}")
                nc.tensor.matmul(stb_ps[:, 0, csl], lhsT=orow,
                                 rhs=s_ap[:, csl].bitcast(mybir.dt.float32r),
                                 start=True, stop=True)
                nc.tensor.matmul(stb_ps[:, 1, csl], lhsT=orow,
                                 rhs=t_ap[:, csl].bitcast(mybir.dt.float32r),
                                 start=True, stop=True)
                st_bc = sb.tile([128, 2, R], BF16, tag="stbc", bufs=1, name=f"stbc_{nm}")
                nc.vector.tensor_copy(st_bc[:, :, csl], stb_ps[:, :, csl])

                xn = sb.tile([128, NC_IN, R], BF16, tag="xn", bufs=3, name=f"xn_{nm}")
                for c in range(NC_IN):
                    nc.vector.tensor_mul(xn[:, c, csl], x_sb[:, c, csl],
                                         st_bc[:, 0, csl])
                    nc.vector.tensor_add(xn[:, c, csl], xn[:, c, csl],
                                         st_bc[:, 1, csl])

                # silu via exp route (phase 3; avoids a table switch)
                if silu_exp:
                    e = ubp.tile([128, NC_IN, R], BF16, tag="u", name=f"se_{nm}")
                    nc.scalar.activation(e[:, :, csl], x_sb[:, :, csl],
                                         AF.Exp, scale=-1.0)
                    with nc.allow_low_precision(reason="sigmoid gate bf16"):
                        nc.vector.tensor_scalar(e[:, :, csl], e[:, :, csl],
                                                1.0, None, OP.add)
                        nc.vector.reciprocal(e[:, :, csl], e[:, :, csl])
                    so = silus[nm]
                    nc.vector.tensor_mul(so[:, :, csl], x_sb[:, :, csl],
                                         e[:, :, csl])
                return xn

            # ---------------- prep stage B: seeds + chain + f8 conversion
            def prep_b(xn, nm, want_f8=True, cols=None, reuse=None):
                lo_, hi_ = cols or (0, R)
                n = hi_ - lo_
                csl = slice(lo_, hi_)
                # seeds: zsq = Square(1.75*xn + 3.5); b0 = Exp(-zsq);
                # rc0 = Exp(3.5*xn + 6).  zsq scratch lives in PSUM (bitcast).
                zsq = ubp.tile([128, NC_IN, R], BF16, tag="u",
                               name=f"zq_{nm}")
                nc.scalar.activation(zsq[:, :, csl], xn[:, :, csl], AF.Square,
                                     scale=1.0 / STEP, bias=b35)
                def new_u(j):
                    if reuse is not None:
                        return reuse["us"][j]
                    if want_f8:
                        return ubp.tile([128, NC_IN, R], BF16, tag="u",
                                        name=f"u{j}_{nm}")
                    return ubop.tile([128, NC_IN, R], BF16, tag="ub8",
                                     name=f"u{j}_{nm}")

                zsq4 = ubp.tile([128, NC_IN, R], BF16, tag="u",
                                name=f"zq4_{nm}")
                nc.scalar.activation(zsq4[:, :, csl], xn[:, :, csl], AF.Square,
                                     scale=1.0 / STEP, bias=bm05)
                rc_prev = sb.tile([128, NC_IN, R], BF16, tag="rc", bufs=3,
                                  name=f"rc0_{nm}")
                nc.scalar.activation(rc_prev[:, :, csl], xn[:, :, csl],
                                     AF.Exp, scale=2.0 / STEP, bias=b60)
                rc4 = sb.tile([128, NC_IN, R], BF16, tag="rc", bufs=3,
                              name=f"rc4_{nm}")
                nc.scalar.activation(rc4[:, :, csl], xn[:, :, csl],
                                     AF.Exp, scale=2.0 / STEP, bias=bm20)
                us = [new_u(0)]
                nc.scalar.activation(us[0][:, :, csl], zsq[:, :, csl],
                                     AF.Exp, scale=-1.0)

                basis8 = None
                if want_f8:
                    basis8 = sb.tile([128, G, 2, 2, R], F8, tag="b8",
                                     bufs=3, name=f"b8_{nm}")

                def conv(u_t, j):
                    if not want_f8:
                        return
                    dst = basis8[:, j, :, :, csl]
                    src = u_t[:, :, csl].rearrange("p (cp ko) r -> p cp ko r",
                                                   cp=2)
                    if j in (3, 7):
                        nc.gpsimd.tensor_copy(dst, src)
                    else:
                        nc.vector.tensor_copy(dst, src)

                conv(us[0], 0)
                # chain A: j = 1..3 from u0
                for j in range(1, 4):
                    us.append(new_u(j))
                    nc.vector.tensor_mul(us[j][:, :, csl],
                                         us[j - 1][:, :, csl],
                                         rc_prev[:, :, csl])
                    conv(us[j], j)
                    if j < 3:
                        rc_t = sb.tile([128, NC_IN, R], BF16, tag="rc", bufs=3,
                                       name=f"rc{j}_{nm}")
                        nc.vector.tensor_mul(rc_t[:, :, csl],
                                             rc_prev[:, :, csl],
                                             rho1[:, :, csl].to_broadcast(
                                                 (128, NC_IN, n)))
                        rc_prev = rc_t
                # chain B: seed u4 (from zsq4), then j = 5 (+6,7 for bf16)
                us.append(new_u(4))
                nc.scalar.activation(us[4][:, :, csl], zsq4[:, :, csl],
                                     AF.Exp, scale=-1.0)
                conv(us[4], 4)
                jend = 7
                for j in range(5, jend + 1):
                    us.append(new_u(j))
                    nc.vector.tensor_mul(us[j][:, :, csl],
                                         us[j - 1][:, :, csl],
                                         rc4[:, :, csl])
                    conv(us[j], j)
                    if j < jend:
                        rc_t = sb.tile([128, NC_IN, R], BF16, tag="rc", bufs=3,
                                       name=f"rc4{j}_{nm}")
                        nc.vector.tensor_mul(rc_t[:, :, csl],
                                             rc4[:, :, csl],
                                             rho1[:, :, csl].to_broadcast(
                                                 (128, NC_IN, n)))
                        rc4 = rc_t
                return {"b8": basis8, "us": us}

            # ---------------- fp8 layer matmuls + epilogues
            DMA_ENGS = (nc.sync, nc.scalar, nc.gpsimd)

            def mm_f8(lname, st, silu, epi):
                for mt in range(2):
                    mm = ps_mm.tile([128, 2, R], F32, tag="mm",
                                    name=f"mm_{lname}{mt}")
                    for mi in range(2):
                        m = 2 * mt + mi
                        wt8 = wtp.tile([128, 16, 2, 128], F8, tag="wt8")
                        nc.sync.dma_start(wt8, io[lname + "_sw8"][m])
                        bwt = wtp.tile([128, NC_IN, 128], BF16, tag="bwt")
                        nc.sync.dma_start(
                            bwt, io[lname + "_bwp"][:, :,
                                                    128 * m:128 * (m + 1)]
                            .rearrange("c p m -> p c m"))
                        for pair in range(16):
                            nc.tensor.matmul(
                                mm[:, mi, :], lhsT=wt8[:, pair, :, :],
                                rhs=st["b8"][:, pair // 2, pair % 2, :, :],
                                start=(pair == 0), stop=False,
                                perf_mode=mybir.MatmulPerfMode.DoubleRow)
                        for c in range(NC_IN):
                            nc.tensor.matmul(
                                mm[:, mi, :], lhsT=bwt[:, c, :],
                                rhs=silu[:, c, :],
                                start=False, stop=(c == NC_IN - 1))
                    epi(mm, mt)

            def epi_qk(ttype, scale):
                def _e(mm, mt):
                    eo = sb.tile([128, 2, R], F8, tag="eo8",
                                 name=f"eoqk{ttype}{mt}")
                    nc.scalar.activation(eo, mm, AF.Identity, scale=scale)
                    for mi in range(2):
                        nc.scalar.dma_start(
                            a_qk_i[4 * mt + 2 * mi:4 * mt + 2 * mi + 2, ttype],
                            eo[:, mi, :].rearrange("(h2 d) r -> h2 d r", h2=2))
                return _e

            def epi_sg(scale):
                def _e(mm, mt):
                    e = sb.tile([128, 2, R], BF16, tag="eob", bufs=1, name=f"eosg{mt}")
                    nc.scalar.activation(e, mm, AF.Exp, scale=-scale)
                    with nc.allow_low_precision(reason="sigmoid gate bf16"):
                        nc.gpsimd.tensor_scalar(e, e, 1.0, None, OP.add)
                        nc.vector.reciprocal(e, e)
                    for mi in range(2):
                        nc.scalar.dma_start(
                            a_sg_i[4 * mt + 2 * mi:4 * mt + 2 * mi + 2],
                            e[:, mi, :].rearrange("(h2 d) r -> h2 d r", h2=2))
                return _e

            def epi_wv(scale):
                def _e(mm, mt):
                    eo = sb.tile([128, 2, R], F8, tag="eo8", name=f"eowv{mt}")
                    nc.scalar.activation(eo, mm, AF.Identity, scale=scale)
                    # transpose [64,128] blocks -> [rows, d] and ship
                    for mi in range(2):
                        for h2 in range(2):
                            tp = ps_mm.tile([128, 2, R], F32, tag="mm",
                                            name=f"tp{mt}{mi}{h2}")
                            tp8 = tp[:, 0, 0:64].bitcast(F8)
                            tpv = tp8.rearrange("p (rc d) -> p rc d", rc=4)
                            for rc in range(4):
                                nc.tensor.transpose(
                                    tpv[:, rc, :],
                                    eo[64 * h2:64 * h2 + 64, mi,
                                       128 * rc:128 * rc + 128],
                                    ident8[64 * h2:64 * h2 + 64,
                                           64 * h2:64 * h2 + 64])
                            stg = sb.tile([128, 4, D], F8, tag="wvstg",
                                          name=f"wvstg{mt}{mi}{h2}")
                            nc.vector.tensor_copy(stg, tpv)
                            nc.scalar.dma_start(
                                a_wv_i[2 * (2 * mt + mi) + h2]
                                .rearrange("(rc p) d -> p rc d", rc=4),
                                stg)
                return _e

            # ---------------- phase 1 schedule
            sts = stats_batch([xk, xq, xv])
            xn_k = prep_a(xk, "k", stats=sts[0])
            xn_q = prep_a(xq, "q", stats=sts[1])
            xn_v = prep_a(xv, "v", stats=sts[2])
            st_k = prep_b(xn_k, "k")
            st_q = prep_b(xn_q, "q")
            st_v = prep_b(xn_v, "v")
            mm_f8("lk", st_k, silus["k"], epi_qk(1, 1.0 / ws["lk"]))
            mm_f8("lv", st_v, silus["v"], epi_wv(1.0 / ws["lv"]))
            a2a(a_wv_i, a_wv_o)
            mm_f8("lq", st_q, silus["q"], epi_qk(0, 1.0 / ws["lq"]))
            a2a(a_qk_i, a_qk_o)
            mm_f8("lg", st_q, silus["q"], epi_sg(1.0 / ws["lg"]))
            a2a(a_sg_i, a_sg_o)

            # ---------------- phase 2 receive tiles
            wqb, wkb, wva, sgb = [], [], [], []
            wq_pk = wk_pk = None
            if stop > 1:
                wq_pk = sb.tile([64, 2, L], F8, tag="wqpk", bufs=1)
                wk_pk = sb.tile([64, 2, L], F8, tag="wkpk", bufs=1)
                engs = (nc.sync, nc.scalar, nc.gpsimd, nc.sync)
                for b in range(B):
                    for t, ty in ((wq_pk, 0), (wk_pk, 1)):
                        for s in range(4):
                            engs[s].dma_start(
                                t[32 * b:32 * b + 32, :,
                                  512 * s:512 * (s + 1)],
                                a_qk_o[4 * b + s, ty]
                                .rearrange("(ko ki) r -> ki ko r", ko=2))
                    wqb.append(wq_pk[32 * b:32 * b + 32, :, :])
                    wkb.append(wk_pk[32 * b:32 * b + 32, :, :])
            for b in range(B) if stop > 1 else []:
                t = sb.tile([128, 8, 2, D + 1], F8, tag=f"wva{b}", bufs=1)
                nc.vector.memset(t[:, :, :, D:D + 1], 1.0)
                for s in range(4):
                    (nc.gpsimd if s % 2 else nc.scalar).dma_start(
                        t[:, 2 * s:2 * s + 2, :, 0:D],
                        a_wv_o[4 * b + s].rearrange(
                            "(pr par p) d -> p pr par d", pr=2, par=2))
                wva.append(t)
                t = sb.tile([D, L], BF16, tag=f"sgb{b}", bufs=1)
                for s in range(4):
                    (nc.scalar if s % 2 else nc.sync).dma_start(
                        t[:, 512 * s:512 * (s + 1)], a_sg_o[4 * b + s])
                sgb.append(t)

            # ---------------- phase 2/3 interleaved
            x3 = sb.tile([128, NC_IN, R], BF16, tag="x", name="x3")
            st_o = None

            def load_x3(hq, src):
                engs = (nc.sync, nc.scalar, nc.gpsimd, nc.sync)
                for c in range(NC_IN):
                    for h2 in range(2):
                        engs[c].dma_start(
                            x3[64 * h2:64 * h2 + 64, c,
                               256 * hq:256 * hq + 256],
                            src[2 * c + h2])

            def lo_mms(mt, cols, mm):
                lo_, hi_ = cols
                csl = slice(lo_, hi_)
                for mi in range(2):
                    m = 2 * mt + mi
                    bwt = wtp.tile([128, NC_IN, 128], BF16, tag="bwt")
                    nc.sync.dma_start(
                        bwt, io["lo_bwp"][:, :, 128 * m:128 * (m + 1)]
                        .rearrange("c p m -> p c m"))
                    for kh in range(4):
                        wt = wtp.tile([128, 8, 128], BF16, tag="wtlo",
                                      bufs=2)
                        DMA_ENGS[kh % 3].dma_start(
                            wt, io["lo_swp"][2 * kh:2 * kh + 2, :, :,
                                             128 * m:128 * (m + 1)]
                            .rearrange("j c i m -> i (j c) m"))
                        for kk in range(8):
                            kc = 8 * kh + kk
                            nc.tensor.matmul(
                                mm[:, mi, csl], lhsT=wt[:, kk, :],
                                rhs=st_o["us"][kc // NC_IN][:, kc % NC_IN, csl],
                                start=(kc == 0), stop=False)
                    for c in range(NC_IN):
                        nc.tensor.matmul(mm[:, mi, csl],
                                         lhsT=bwt[:, c, :],
                                         rhs=silus["o"][:, c, csl],
                                         start=False, stop=(c == NC_IN - 1))

            lo_mm_tiles = {}

            for qc in range(NQC) if stop > 1 else []:
                qsl = slice(QC * qc, QC * (qc + 1))
                av_t = ps_mm.tile([128, 2, QC], F32, tag="mm",
                                  name=f"av{qc}")
                av = av_t[0:D + 1, :, :]
                a8_t = None
                for kt in range(NKT):
                    S = ps_s.tile([128, 2, QC], F32, tag="S", name=f"S{qc}_{kt}")
                    for b in range(B):
                        nc.tensor.matmul(
                            S[:, b, :],
                            lhsT=wkb[b][:, :, 128 * kt:128 * (kt + 1)],
                            rhs=wqb[b][:, :, qsl],
                            start=True, stop=True,
                            perf_mode=mybir.MatmulPerfMode.DoubleRow)
                    if kt % 2 == 0:
                        a8_t = sb.tile([128, 2, 2, QC], F8, tag="a8",
                                       name=f"a8_{qc}_{kt // 2}")
                    nc.scalar.activation(a8_t[:, kt % 2, :, :], S, AF.Exp)
                    if kt % 2 == 1:
                        for b in range(B):
                            nc.tensor.matmul(
                                av[:, b, :],
                                lhsT=wva[b][:, kt // 2, :, :],
                                rhs=a8_t[:, :, b, :],
                                start=(kt == 1), stop=(kt == NKT - 1),
                                perf_mode=mybir.MatmulPerfMode.DoubleRow)
                # gating: og = av[0:D] * (1/den) * sg
                rcpb = sb.tile([1, 2, QC], BF16, tag="rcpb", bufs=1,
                               name=f"rcpb{qc}")
                with nc.allow_low_precision(reason="softmax denom bf16"):
                    nc.vector.reciprocal(rcpb, av[D:D + 1, :, :])
                rb = ps_s.tile([128, 2, QC], F32, tag="S", name=f"rb{qc}")
                for b in range(B):
                    nc.tensor.matmul(rb[0:D, b, :],
                                     lhsT=ones_row[0:1, 0:D],
                                     rhs=rcpb[:, b, :], start=True, stop=True)
                og = sb.tile([D, 2, QC], BF16, tag="avs", bufs=1,
                             name=f"og{qc}")
                for b in range(B):
                    nc.vector.tensor_mul(og[:, b, :], av[0:D, b, :],
                                         sgb[b][:, qsl])
                nc.vector.scalar_tensor_tensor(og, og, 1.0, rb[0:D, :, :],
                                               OP.mult, OP.mult)
                half = qc // 2
                dstbuf = a_oA_i if half == 0 else a_oB_i
                for b in range(B):
                    for hh in range(2):
                        (nc.sync if hh else nc.scalar).dma_start(
                            dstbuf[4 * b + 2 * (qc % 2) + hh],
                            og[:, b, 256 * hh:256 * hh + 256])

                # interleave phase-3 work (staged to keep the exp stream hot)
                if qc == 1:
                    a2a(a_oA_i, a_oA_o)
                    load_x3(0, a_oA_o)
                    sts_oA = stats_batch([x3], cols=(0, 256))
                    silus["o"] = sb3.tile([128, NC_IN, R], BF16, tag="silu",
                                          name="silu_o")
                    xn_oA = prep_a(x3, "o", cols=(0, 256), silu_exp=True,
                                   stats=sts_oA[0])
                if qc == 2:
                    st_o = prep_b(xn_oA, "o", want_f8=False, cols=(0, 256))
                if qc == 3:
                    lo_mm_tiles[0] = ps_mm.tile([128, 2, R], F32, tag="mm",
                                                name="mm_lo0")
                    lo_mms(0, (0, 256), lo_mm_tiles[0])
                    a2a(a_oB_i, a_oB_o)
                    load_x3(1, a_oB_o)

            # phase-3 tail
            if stop > 1:
                xn_oB = prep_a(x3, "o", cols=(256, R), silu_exp=True)
                prep_b(xn_oB, "o", want_f8=False, cols=(256, R),
                       reuse=st_o)
                lo_mms(0, (256, R), lo_mm_tiles[0])
                eo = sb.tile([128, 2, R], BF16, tag="eof", bufs=1,
                             name="eo_out0")
                nc.scalar.activation(eo, lo_mm_tiles[0], AF.Identity)
                nc.gpsimd.dma_start(io["outT"][0], eo)
                mm1 = ps_s.tile([128, 2, R], F32, tag="S", name="mm_lo1")
                lo_mms(1, (0, R), mm1)
                eo1 = sb.tile([128, 2, R], BF16, tag="eof", bufs=1,
                              name="eo_out1")
                nc.scalar.activation(eo1, mm1, AF.Identity)
                nc.gpsimd.dma_start(io["outT"][1], eo1)

    nc.compile()
    return nc


# ------------------------------------------------------------------------- host
def _f8(x):
    return np.clip(np.asarray(x, np.float32), -448, 448).astype(F8NP)


def _bf(x):
    return np.asarray(x, np.float32).astype(BFNP)


def _prep_weights(inputs):
    w = {}
    ws = {}
    for l, sc in (("lq", float(D) ** -0.5), ("lk", 1.0), ("lv", 1.0),
                  ("lg", 1.0), ("lo", 1.0)):
        sw = np.asarray(inputs[l + "_sw"], np.float32) * sc
        bw = np.asarray(inputs[l + "_bw"], np.float32) * sc
        assert np.allclose(np.asarray(inputs[l + "_bb"]), 0.0), "bias != 0"
        assert np.all(np.asarray(inputs[l + "_ln_s"]) == 1.0)
        assert np.all(np.asarray(inputs[l + "_ln_b"]) == 0.0)
        if l == "lo":
            swp = _bf(sw.reshape(OUT, NC_IN, 128, G).transpose(3, 1, 2, 0))
            w["lo_swp"] = np.ascontiguousarray(swp)
            w["lo_bwp"] = np.ascontiguousarray(_bf(bw.T.reshape(NC_IN, 128, OUT)))
            ws[l] = 1.0
        else:
            s = 2.0 ** np.floor(np.log2(112.0 / np.abs(sw).max()))
            ws[l] = float(s)
            # sw [out, in*G]; in = c*128+p, c = 2*cp+ko -> [pair=(j,cp),p,ko,out]
            sw_r = (sw * s).reshape(4, 128, 2, 2, 128, G)  # [m,mc,cp,ko,p,j]
            sw8 = sw_r.transpose(0, 4, 5, 2, 3, 1).reshape(4, 128, 16, 2, 128)
            w[l + "_sw8"] = np.ascontiguousarray(_f8(sw8))
            w[l + "_bwp"] = np.ascontiguousarray(
                _bf((bw * s).T.reshape(NC_IN, 128, OUT)))
    return w, ws


def kernel(**inputs):
    w, ws = _prep_weights(inputs)
    key = tuple(sorted(ws.items()))
    if _cache.get("key") != key:
        _cache["nc"] = _build_program(ws)
        _cache["key"] = key
    nc = _cache["nc"]

    q = np.asarray(inputs["q"], np.float32).reshape(B * L, IN)
    k = np.asarray(inputs["k"], np.float32).reshape(B * L, IN)
    v = np.asarray(inputs["v"], np.float32).reshape(B * L, IN)

    in_maps = []
    for core in range(NCORES):
        rows = slice(R * core, R * (core + 1))
        xT3 = np.stack([np.ascontiguousarray(_bf(q[rows].T)),
                        np.ascontiguousarray(_bf(k[rows].T)),
                        np.ascontiguousarray(_bf(v[rows].T))])
        m = {"xT3": xT3}
        m.update(w)
        in_maps.append(m)

    trace = bool(int(os.environ.get("KERNEL_TRACE", "0")))
    res = run_bass_kernel_spmd(nc, in_maps, core_ids=list(range(NCORES)),
                               trace=trace)
    _cache["last_result"] = res

    # unshard: core r holds batch r//4, q ranges [(r%4)*256, +256) and
    # [1024+(r%4)*256, +256); outT [2(m-big), 128, 2(mi), R]
    out = np.zeros((B, L, OUT), np.float32)
    for core in range(NCORES):
        o = res.results[core]["outT"].reshape(2, 128, 2, R)
        o = o.transpose(0, 2, 1, 3).reshape(OUT, R)   # [outdim, rows]
        b = core // 4
        q0 = (core % 4) * 256
        out[b, q0:q0 + 256, :] = o[:, 0:256].T
        out[b, 1024 + q0:1024 + q0 + 256, :] = o[:, 256:R].T
    return out


# revision 55
# speedup vs baseline: 1.0325x; 1.0325x over previous
"""AttentionWithFastKANTransform Trainium2 kernel (8 NeuronCores, SPMD).

v2 design:
  phase 1 (row-sharded, R=512 rows/core): FastKAN lq/lk/lv/lg with fp8
    DoubleRow spline matmuls (4x fewer PE cycles). RBF basis built by a
    bf16 multiply chain on DVE (b_{j+1} = b_j * rc_j, rc_{j+1} = rc_j*e^-2)
    seeded by two ACT exps, then converted to fp8 tiles for the matmuls.
  AllToAlls (fp8): wq/wk -> [32,2ko,L] per head; wv locally PE-transposed
    to [k,d] before the a2a; sigmoid gate bf16.
  phase 2 (head-sharded): S = wk^T wq fp8 DoubleRow (k-partitioned, 32x2
    contraction), exp on ACT -> fp8 A pair-tiles, att@V fp8 DoubleRow over
    k-tile pairs with a ones-column for softmax denominators.
  Gated output a2a'd back in two halves (bf16) so phase 3 overlaps phase 2.
  phase 3: FastKAN lo with bf16 spline (fp8 too lossy for the final layer),
    split in two row-halves for overlap.
"""

import os
import numpy as np
import ml_dtypes

import concourse.bass as bass
import concourse.bacc as bacc
import concourse.tile as tile
import concourse.mybir as mybir
from concourse.bass_utils import run_bass_kernel_spmd
from concourse.masks import make_identity

AF = mybir.ActivationFunctionType
OP = mybir.AluOpType
F32 = mybir.dt.float32
BF16 = mybir.dt.bfloat16
F8 = mybir.dt.float8e4
F8NP = ml_dtypes.float8_e4m3fn
BFNP = ml_dtypes.bfloat16

NCORES = 8
B, L, IN, OUT, H, D, G = 2, 2048, 512, 512, 8, 64, 8
R = (B * L) // NCORES          # 512 rows per core
NC_IN = IN // 128              # 4 input-dim chunks
NKT = L // 128                 # 16 k-tiles per batch
STEP = 4.0 / (G - 1)
EPS = 1e-5
QC = 512                       # phase-2 q-chunk
NQC = L // QC                  # 4
F8_LAYERS = ("lq", "lk", "lv", "lg")
RHO = float(np.exp(-2.0))

_cache = {}


def _patch_act_tables():
    """Prefer the ln+exp combined table so Ln/Exp don't ping-pong loads."""
    if _cache.get("act_patched"):
        return
    import concourse.bacc as _bacc
    import concourse.hw_specs as _hw
    orig = _hw.get_activation_tables

    def reordered(arch):
        t = dict(orig(arch))
        order = ["natural_log_exp_and_others", "silu_and_others"]
        out = {k: t[k] for k in order if k in t}
        out.update({k: v for k, v in t.items() if k not in out})
        return out

    _bacc.get_activation_tables = reordered
    _cache["act_patched"] = True


def _build_program(ws):
    """ws: dict layer -> fp8 weight scale (host-derived, baked as consts)."""
    _patch_act_tables()
    nc = bacc.Bacc("TRN2", target_bir_lowering=False, debug=False,
                   num_devices=NCORES)
    io = {}
    io["xT3"] = nc.dram_tensor("xT3", [3, IN, R], BF16, kind="ExternalInput").ap()
    for l in F8_LAYERS:
        io[l + "_sw8"] = nc.dram_tensor(l + "_sw8", [4, 128, 16, 2, 128], F8,
                                        kind="ExternalInput").ap()
    io["lo_swp"] = nc.dram_tensor("lo_swp", [G, NC_IN, 128, OUT], BF16,
                                  kind="ExternalInput").ap()
    for l in F8_LAYERS + ("lo",):
        io[l + "_bwp"] = nc.dram_tensor(l + "_bwp", [NC_IN, 128, OUT], BF16,
                                        kind="ExternalInput").ap()
    io["outT"] = nc.dram_tensor("outT", [2, 128, 2, R], BF16,
                                kind="ExternalOutput").ap()

    rg = [list(range(NCORES))]
    nocc = bool(int(os.environ.get("KERNEL_NOCC", "0")))
    stop = int(os.environ.get("KERNEL_STOP", "9"))

    with tile.TileContext(nc) as tc:
        with tc.tile_pool(name="dram1", bufs=1, space="DRAM") as dram1, \
             tc.tile_pool(name="sb", bufs=2) as sb, \
             tc.tile_pool(name="sb3", bufs=3) as sb3, \
             tc.tile_pool(name="ub", bufs=3) as ubp, \
             tc.tile_pool(name="ubo", bufs=8) as ubop, \
             tc.tile_pool(name="wt", bufs=2) as wtp, \
             tc.tile_pool(name="consts", bufs=1) as cpool, \
             tc.tile_pool(name="ps_mm", bufs=2, space="PSUM") as ps_mm, \
             tc.tile_pool(name="ps_s", bufs=2, space="PSUM") as ps_s:

            # ---------------- collective buffers
            a_qk_i = dram1.tile([NCORES, 2, D, R], F8, tag="aqk_i")
            a_qk_o = dram1.tile([NCORES, 2, D, R], F8, tag="aqk_o")
            a_sg_i = dram1.tile([NCORES, D, R], BF16, tag="asg_i")
            a_sg_o = dram1.tile([NCORES, D, R], BF16, tag="asg_o")
            a_wv_i = dram1.tile([NCORES, R, D], F8, tag="awv_i")
            a_wv_o = dram1.tile([NCORES, R, D], F8, tag="awv_o")
            a_oA_i = dram1.tile([NCORES, D, R // 2], BF16, tag="aoA_i")
            a_oA_o = dram1.tile([NCORES, D, R // 2], BF16, tag="aoA_o")
            a_oB_i = dram1.tile([NCORES, D, R // 2], BF16, tag="aoB_i")
            a_oB_o = dram1.tile([NCORES, D, R // 2], BF16, tag="aoB_o")

            def a2a(i, o):
                if nocc:
                    nc.sync.dma_start(o, i)
                else:
                    nc.gpsimd.collective_compute(
                        "AllToAll", OP.bypass, replica_groups=rg,
                        ins=[i.opt()], outs=[o.opt()])

            # ---------------- consts
            ones_col = cpool.tile([128, 1], BF16, tag="ones_col")
            nc.vector.memset(ones_col, 1.0 / IN)
            ones_row = cpool.tile([128, 128], BF16, tag="ones_row")
            nc.vector.memset(ones_row, 1.0)
            ones_rowf = cpool.tile([128, 128], F32, tag="ones_rowf")
            nc.vector.memset(ones_rowf, 1.0)
            ident8 = cpool.tile([128, 128], F8, tag="ident8")
            make_identity(nc, ident8)
            rho1 = cpool.tile([128, 1, R], BF16, tag="rho")
            nc.vector.memset(rho1, RHO)
            bm25 = cpool.tile([128, 1], F32, tag="bm25")
            nc.vector.memset(bm25, 3.5 - 6.0)
            bm35 = cpool.tile([128, 1], F32, tag="bm35")
            nc.vector.memset(bm35, 3.5 - 7.0)
            bm05 = cpool.tile([128, 1], F32, tag="bm05")
            nc.vector.memset(bm05, 3.5 - 4.0)
            bm20 = cpool.tile([128, 1], F32, tag="bm20")
            nc.vector.memset(bm20, 6.0 - 8.0)
            epst = cpool.tile([128, 1], F32, tag="eps")
            nc.vector.memset(epst, EPS)
            b35 = cpool.tile([128, 1], F32, tag="b35")
            nc.vector.memset(b35, 3.5)
            b60 = cpool.tile([128, 1], F32, tag="b60")
            nc.vector.memset(b60, 6.0)

            # PE warmup: keep the array busy during input DMA so the
            # first real matmuls run at full clock
            warm_ps = ps_s.tile([128, 2, R], F32, tag="S", name="warm")
            for w in range(10):
                nc.tensor.matmul(warm_ps[:, 0, 0:128], lhsT=ones_row,
                                 rhs=ones_row, start=True, stop=True,
                                 skip_group_check=True)

            # ---------------- x loads + batched silus (one table switch)
            def load_x(idx):
                x = sb3.tile([128, NC_IN, R], BF16, tag="x", name=f"x{idx}")
                nc.sync.dma_start(
                    x, io["xT3"][idx].rearrange("(c p) r -> p c r", p=128))
                return x

            xk, xq, xv = load_x(1), load_x(0), load_x(2)
            silus = {}
            for nm, x in (("k", xk), ("q", xq), ("v", xv)):
                s = sb3.tile([128, NC_IN, R], BF16, tag="silu", name=f"silu_{nm}")
                nc.scalar.activation(s, x, AF.Silu)
                silus[nm] = s

            # ---------------- batched LN stats (k,q,v in one Ln/Exp pair)
            def stats_batch(xs, cols=None):
                """xs: list of (x_sb, xsq_writer) tensors; returns list of
                (s_ap, t_ap) [1, n] access patterns per tensor."""
                lo_, hi_ = cols or (0, R)
                n = hi_ - lo_
                csl = slice(lo_, hi_)
                nt = len(xs)
                stat = ps_s.tile([97, 2, R], F32, tag="S", name="statb")
                for t, x_sb in enumerate(xs):
                    xsq = ubp.tile([128, NC_IN, R], BF16, tag="u",
                                   name=f"xsqb{t}")
                    for c in range(NC_IN):
                        nc.tensor.matmul(stat[32 * t:32 * t + 1, 0, csl],
                                         lhsT=ones_col, rhs=x_sb[:, c, csl],
                                         start=(c == 0), stop=(c == NC_IN - 1))
                    nc.vector.tensor_mul(xsq[:, :, csl], x_sb[:, :, csl],
                                         x_sb[:, :, csl])
                    for c in range(NC_IN):
                        nc.tensor.matmul(stat[32 * t:32 * t + 1, 1, csl],
                                         lhsT=ones_col, rhs=xsq[:, c, csl],
                                         start=(c == 0), stop=(c == NC_IN - 1))
                # stat rows now hold mu and E[x^2] directly (1/IN folded)
                sm = sb.tile([97, 3, R], F32, tag="stsm", bufs=1, name="smb")
                nc.gpsimd.memset(sm, 1.0)
                for t in range(nt):
                    p = slice(32 * t, 32 * t + 1)
                    var = sm[p, 1, csl]
                    # mumu = mu^2 (one PSUM operand is allowed)
                    nc.vector.scalar_tensor_tensor(sm[p, 2, csl],
                                                   stat[p, 0, csl], 1.0,
                                                   stat[p, 0, csl],
                                                   OP.mult, OP.mult)
                    nc.vector.tensor_sub(var, stat[p, 1, csl],
                                         sm[p, 2, csl])
                # one Ln + one Exp over all tensors (spread on partitions)
                nc.scalar.activation(sm[:, 2, csl], sm[:, 1, csl],
                                     AF.Ln, bias=epst[0:97])
                nc.scalar.activation(sm[:, 1, csl], sm[:, 2, csl],
                                     AF.Exp, scale=-0.5)
                for t in range(nt):
                    p = slice(32 * t, 32 * t + 1)
                    nc.vector.scalar_tensor_tensor(sm[p, 2, csl],
                                                   stat[p, 0, csl], -1.0,
                                                   sm[p, 1, csl],
                                                   OP.mult, OP.mult)
                return [(sm[32 * t:32 * t + 1, 1, :],
                         sm[32 * t:32 * t + 1, 2, :]) for t in range(nt)]

            # ---------------- prep stage A: xn (+ phase-3 silu)
            def prep_a(x_sb, nm, cols=None, silu_exp=False, stats=None):
                lo_, hi_ = cols or (0, R)
                n = hi_ - lo_
                csl = slice(lo_, hi_)

                if stats is None:
                    stats = stats_batch([x_sb], cols=cols)[0]
                s_ap, t_ap = stats
                # broadcast via PE: [1,n] -> [128,n] (two mms, one per bank)
                bp = s_ap.base_partition()
                orow = ones_rowf[bp:bp + 1, :].bitcast(mybir.dt.float32r)
                stb_ps = ps_s.tile([128, 2, R], F32, tag="S", name=f"stb_{nm}")
                nc.tensor.matmul(stb_ps[:, 0, csl], lhsT=orow,
                                 rhs=s_ap[:, csl].bitcast(mybir.dt.float32r),
                                 start=True, stop=True)
                nc.tensor.matmul(stb_ps[:, 1, csl], lhsT=orow,
                                 rhs=t_ap[:, csl].bitcast(mybir.dt.float32r),
                                 start=True, stop=True)
                st_bc = sb.tile([128, 2, R], BF16, tag="stbc", bufs=1, name=f"stbc_{nm}")
                nc.vector.tensor_copy(st_bc[:, :, csl], stb_ps[:, :, csl])

                xn = sb.tile([128, NC_IN, R], BF16, tag="xn", bufs=3, name=f"xn_{nm}")
                for c in range(NC_IN):
                    nc.vector.tensor_mul(xn[:, c, csl], x_sb[:, c, csl],
                                         st_bc[:, 0, csl])
                    nc.vector.tensor_add(xn[:, c, csl], xn[:, c, csl],
                                         st_bc[:, 1, csl])

                # silu via exp route (phase 3; avoids a table switch)
                if silu_exp:
                    e = ubp.tile([128, NC_IN, R], BF16, tag="u", name=f"se_{nm}")
                    nc.scalar.activation(e[:, :, csl], x_sb[:, :, csl],
                                         AF.Exp, scale=-1.0)
                    with nc.allow_low_precision(reason="sigmoid gate bf16"):
                        nc.vector.tensor_scalar(e[:, :, csl], e[:, :, csl],
                                                1.0, None, OP.add)
                        nc.vector.reciprocal(e[:, :, csl], e[:, :, csl])
                    so = silus[nm]
                    nc.vector.tensor_mul(so[:, :, csl], x_sb[:, :, csl],
                                         e[:, :, csl])
                return xn

            # ---------------- prep stage B: seeds + chain + f8 conversion
            def prep_b(xn, nm, want_f8=True, cols=None, reuse=None):
                lo_, hi_ = cols or (0, R)
                n = hi_ - lo_
                csl = slice(lo_, hi_)
                # seeds: zsq = Square(1.75*xn + 3.5); b0 = Exp(-zsq);
                # rc0 = Exp(3.5*xn + 6).  zsq scratch lives in PSUM (bitcast).
                def psum_bf16(name):
                    t = ps_s.tile([128, 2, R], F32, tag="S", name=name)
                    return t.bitcast(BF16).rearrange(
                        "p a (b r) -> p (a b) r", b=2)
                zsq = psum_bf16(f"zq_{nm}")
                nc.scalar.activation(zsq[:, :, csl], xn[:, :, csl], AF.Square,
                                     scale=1.0 / STEP, bias=b35)
                def new_u(j):
                    if reuse is not None:
                        return reuse["us"][j]
                    if want_f8:
                        return ubp.tile([128, NC_IN, R], BF16, tag="u",
                                        name=f"u{j}_{nm}")
                    return ubop.tile([128, NC_IN, R], BF16, tag="ub8",
                                     name=f"u{j}_{nm}")

                zsq4 = psum_bf16(f"zq4_{nm}")
                nc.scalar.activation(zsq4[:, :, csl], xn[:, :, csl], AF.Square,
                                     scale=1.0 / STEP, bias=bm05)
                rc_prev = sb.tile([128, NC_IN, R], BF16, tag="rc", bufs=3,
                                  name=f"rc0_{nm}")
                nc.scalar.activation(rc_prev[:, :, csl], xn[:, :, csl],
                                     AF.Exp, scale=2.0 / STEP, bias=b60)
                rc4 = sb.tile([128, NC_IN, R], BF16, tag="rc", bufs=3,
                              name=f"rc4_{nm}")
                nc.scalar.activation(rc4[:, :, csl], xn[:, :, csl],
                                     AF.Exp, scale=2.0 / STEP, bias=bm20)
                us = [new_u(0)]
                nc.scalar.activation(us[0][:, :, csl], zsq[:, :, csl],
                                     AF.Exp, scale=-1.0)

                basis8 = None
                if want_f8:
                    basis8 = sb.tile([128, G, 2, 2, R], F8, tag="b8",
                                     bufs=3, name=f"b8_{nm}")

                def conv(u_t, j):
                    if not want_f8:
                        return
                    dst = basis8[:, j, :, :, csl]
                    src = u_t[:, :, csl].rearrange("p (cp ko) r -> p cp ko r",
                                                   cp=2)
                    if j in (3, 7):
                        nc.gpsimd.tensor_copy(dst, src)
                    else:
                        nc.vector.tensor_copy(dst, src)

                conv(us[0], 0)
                # chain A: j = 1..3 from u0
                for j in range(1, 4):
                    us.append(new_u(j))
                    nc.vector.tensor_mul(us[j][:, :, csl],
                                         us[j - 1][:, :, csl],
                                         rc_prev[:, :, csl])
                    conv(us[j], j)
                    if j < 3:
                        rc_t = sb.tile([128, NC_IN, R], BF16, tag="rc", bufs=3,
                                       name=f"rc{j}_{nm}")
                        nc.vector.tensor_mul(rc_t[:, :, csl],
                                             rc_prev[:, :, csl],
                                             rho1[:, :, csl].to_broadcast(
                                                 (128, NC_IN, n)))
                        rc_prev = rc_t
                # chain B: seed u4 (from zsq4), then j = 5 (+6,7 for bf16)
                us.append(new_u(4))
                nc.scalar.activation(us[4][:, :, csl], zsq4[:, :, csl],
                                     AF.Exp, scale=-1.0)
                conv(us[4], 4)
                jend = 7
                for j in range(5, jend + 1):
                    us.append(new_u(j))
                    nc.vector.tensor_mul(us[j][:, :, csl],
                                         us[j - 1][:, :, csl],
                                         rc4[:, :, csl])
                    conv(us[j], j)
                    if j < jend:
                        rc_t = sb.tile([128, NC_IN, R], BF16, tag="rc", bufs=3,
                                       name=f"rc4{j}_{nm}")
                        nc.vector.tensor_mul(rc_t[:, :, csl],
                                             rc4[:, :, csl],
                                             rho1[:, :, csl].to_broadcast(
                                                 (128, NC_IN, n)))
                        rc4 = rc_t
                return {"b8": basis8, "us": us}

            # ---------------- fp8 layer matmuls + epilogues
            DMA_ENGS = (nc.sync, nc.scalar, nc.gpsimd)

            def mm_f8(lname, st, silu, epi):
                for mt in range(2):
                    mm = ps_mm.tile([128, 2, R], F32, tag="mm",
                                    name=f"mm_{lname}{mt}")
                    for mi in range(2):
                        m = 2 * mt + mi
                        wt8 = wtp.tile([128, 16, 2, 128], F8, tag="wt8")
                        nc.sync.dma_start(wt8, io[lname + "_sw8"][m])
                        bwt = wtp.tile([128, NC_IN, 128], BF16, tag="bwt")
                        nc.sync.dma_start(
                            bwt, io[lname + "_bwp"][:, :,
                                                    128 * m:128 * (m + 1)]
                            .rearrange("c p m -> p c m"))
                        for pair in range(16):
                            nc.tensor.matmul(
                                mm[:, mi, :], lhsT=wt8[:, pair, :, :],
                                rhs=st["b8"][:, pair // 2, pair % 2, :, :],
                                start=(pair == 0), stop=False,
                                perf_mode=mybir.MatmulPerfMode.DoubleRow)
                        for c in range(NC_IN):
                            nc.tensor.matmul(
                                mm[:, mi, :], lhsT=bwt[:, c, :],
                                rhs=silu[:, c, :],
                                start=False, stop=(c == NC_IN - 1))
                    epi(mm, mt)

            def epi_qk(ttype, scale):
                def _e(mm, mt):
                    eo = sb.tile([128, 2, R], F8, tag="eo8",
                                 name=f"eoqk{ttype}{mt}")
                    nc.scalar.activation(eo, mm, AF.Identity, scale=scale)
                    for mi in range(2):
                        nc.scalar.dma_start(
                            a_qk_i[4 * mt + 2 * mi:4 * mt + 2 * mi + 2, ttype],
                            eo[:, mi, :].rearrange("(h2 d) r -> h2 d r", h2=2))
                return _e

            def epi_sg(scale):
                def _e(mm, mt):
                    e = sb.tile([128, 2, R], BF16, tag="eob", bufs=1, name=f"eosg{mt}")
                    nc.scalar.activation(e, mm, AF.Exp, scale=-scale)
                    with nc.allow_low_precision(reason="sigmoid gate bf16"):
                        nc.gpsimd.tensor_scalar(e, e, 1.0, None, OP.add)
                        nc.vector.reciprocal(e, e)
                    for mi in range(2):
                        nc.scalar.dma_start(
                            a_sg_i[4 * mt + 2 * mi:4 * mt + 2 * mi + 2],
                            e[:, mi, :].rearrange("(h2 d) r -> h2 d r", h2=2))
                return _e

            def epi_wv(scale):
                def _e(mm, mt):
                    eo = sb.tile([128, 2, R], F8, tag="eo8", name=f"eowv{mt}")
                    nc.scalar.activation(eo, mm, AF.Identity, scale=scale)
                    # transpose [64,128] blocks -> [rows, d] and ship
                    for mi in range(2):
                        for h2 in range(2):
                            tp = ps_mm.tile([128, 2, R], F32, tag="mm",
                                            name=f"tp{mt}{mi}{h2}")
                            tp8 = tp[:, 0, 0:64].bitcast(F8)
                            tpv = tp8.rearrange("p (rc d) -> p rc d", rc=4)
                            for rc in range(4):
                                nc.tensor.transpose(
                                    tpv[:, rc, :],
                                    eo[64 * h2:64 * h2 + 64, mi,
                                       128 * rc:128 * rc + 128],
                                    ident8[64 * h2:64 * h2 + 64,
                                           64 * h2:64 * h2 + 64])
                            stg = sb.tile([128, 4, D], F8, tag="wvstg",
                                          name=f"wvstg{mt}{mi}{h2}")
                            nc.vector.tensor_copy(stg, tpv)
                            nc.scalar.dma_start(
                                a_wv_i[2 * (2 * mt + mi) + h2]
                                .rearrange("(rc p) d -> p rc d", rc=4),
                                stg)
                return _e

            # ---------------- phase 1 schedule
            sts = stats_batch([xk, xq, xv])
            xn_k = prep_a(xk, "k", stats=sts[0])
            xn_q = prep_a(xq, "q", stats=sts[1])
            xn_v = prep_a(xv, "v", stats=sts[2])
            st_k = prep_b(xn_k, "k")
            st_q = prep_b(xn_q, "q")
            st_v = prep_b(xn_v, "v")
            mm_f8("lk", st_k, silus["k"], epi_qk(1, 1.0 / ws["lk"]))
            mm_f8("lq", st_q, silus["q"], epi_qk(0, 1.0 / ws["lq"]))
            a2a(a_qk_i, a_qk_o)
            mm_f8("lv", st_v, silus["v"], epi_wv(1.0 / ws["lv"]))
            a2a(a_wv_i, a_wv_o)
            mm_f8("lg", st_q, silus["q"], epi_sg(1.0 / ws["lg"]))
            a2a(a_sg_i, a_sg_o)

            # ---------------- phase 2 receive tiles
            wqb, wkb, wva, sgb = [], [], [], []
            wq_pk = wk_pk = None
            if stop > 1:
                wq_pk = sb.tile([64, 2, L], F8, tag="wqpk", bufs=1)
                wk_pk = sb.tile([64, 2, L], F8, tag="wkpk", bufs=1)
                engs = (nc.sync, nc.scalar, nc.gpsimd, nc.sync)
                for b in range(B):
                    for t, ty in ((wq_pk, 0), (wk_pk, 1)):
                        for s in range(4):
                            engs[s].dma_start(
                                t[32 * b:32 * b + 32, :,
                                  512 * s:512 * (s + 1)],
                                a_qk_o[4 * b + s, ty]
                                .rearrange("(ko ki) r -> ki ko r", ko=2))
                    wqb.append(wq_pk[32 * b:32 * b + 32, :, :])
                    wkb.append(wk_pk[32 * b:32 * b + 32, :, :])
            for b in range(B) if stop > 1 else []:
                t = sb.tile([128, 8, 2, D + 1], F8, tag=f"wva{b}", bufs=1)
                nc.vector.memset(t[:, :, :, D:D + 1], 1.0)
                for s in range(4):
                    (nc.gpsimd if s % 2 else nc.scalar).dma_start(
                        t[:, 2 * s:2 * s + 2, :, 0:D],
                        a_wv_o[4 * b + s].rearrange(
                            "(pr par p) d -> p pr par d", pr=2, par=2))
                wva.append(t)
                t = sb.tile([D, L], BF16, tag=f"sgb{b}", bufs=1)
                for s in range(4):
                    (nc.scalar if s % 2 else nc.sync).dma_start(
                        t[:, 512 * s:512 * (s + 1)], a_sg_o[4 * b + s])
                sgb.append(t)

            # ---------------- phase 2/3 interleaved
            x3 = sb.tile([128, NC_IN, R], BF16, tag="x", name="x3")
            st_o = None

            def load_x3(hq, src):
                engs = (nc.sync, nc.scalar, nc.gpsimd, nc.sync)
                for c in range(NC_IN):
                    for h2 in range(2):
                        engs[c].dma_start(
                            x3[64 * h2:64 * h2 + 64, c,
                               256 * hq:256 * hq + 256],
                            src[2 * c + h2])

            def lo_mms(mt, cols, mm):
                lo_, hi_ = cols
                csl = slice(lo_, hi_)
                for mi in range(2):
                    m = 2 * mt + mi
                    bwt = wtp.tile([128, NC_IN, 128], BF16, tag="bwt")
                    nc.sync.dma_start(
                        bwt, io["lo_bwp"][:, :, 128 * m:128 * (m + 1)]
                        .rearrange("c p m -> p c m"))
                    for kh in range(4):
                        wt = wtp.tile([128, 8, 128], BF16, tag="wtlo",
                                      bufs=2)
                        DMA_ENGS[kh % 3].dma_start(
                            wt, io["lo_swp"][2 * kh:2 * kh + 2, :, :,
                                             128 * m:128 * (m + 1)]
                            .rearrange("j c i m -> i (j c) m"))
                        for kk in range(8):
                            kc = 8 * kh + kk
                            nc.tensor.matmul(
                                mm[:, mi, csl], lhsT=wt[:, kk, :],
                                rhs=st_o["us"][kc // NC_IN][:, kc % NC_IN, csl],
                                start=(kc == 0), stop=False)
                    for c in range(NC_IN):
                        nc.tensor.matmul(mm[:, mi, csl],
                                         lhsT=bwt[:, c, :],
                                         rhs=silus["o"][:, c, csl],
                                         start=False, stop=(c == NC_IN - 1))

            lo_mm_tiles = {}

            for qc in range(NQC) if stop > 1 else []:
                qsl = slice(QC * qc, QC * (qc + 1))
                av_t = ps_mm.tile([128, 2, QC], F32, tag="mm",
                                  name=f"av{qc}")
                av = av_t[0:D + 1, :, :]
                a8_t = None
                for kt in range(NKT):
                    S = ps_s.tile([128, 2, QC], F32, tag="S", name=f"S{qc}_{kt}")
                    for b in range(B):
                        nc.tensor.matmul(
                            S[:, b, :],
                            lhsT=wkb[b][:, :, 128 * kt:128 * (kt + 1)],
                            rhs=wqb[b][:, :, qsl],
                            start=True, stop=True,
                            perf_mode=mybir.MatmulPerfMode.DoubleRow)
                    if kt % 2 == 0:
                        a8_t = sb.tile([128, 2, 2, QC], F8, tag="a8",
                                       name=f"a8_{qc}_{kt // 2}")
                    nc.scalar.activation(a8_t[:, kt % 2, :, :], S, AF.Exp)
                    if kt % 2 == 1:
                        for b in range(B):
                            nc.tensor.matmul(
                                av[:, b, :],
                                lhsT=wva[b][:, kt // 2, :, :],
                                rhs=a8_t[:, :, b, :],
                                start=(kt == 1), stop=(kt == NKT - 1),
                                perf_mode=mybir.MatmulPerfMode.DoubleRow)
                # gating: og = av[0:D] * (1/den) * sg
                rcpb = sb.tile([1, 2, QC], BF16, tag="rcpb", bufs=1,
                               name=f"rcpb{qc}")
                with nc.allow_low_precision(reason="softmax denom bf16"):
                    nc.vector.reciprocal(rcpb, av[D:D + 1, :, :])
                rb = ps_s.tile([128, 2, QC], F32, tag="S", name=f"rb{qc}")
                for b in range(B):
                    nc.tensor.matmul(rb[0:D, b, :],
                                     lhsT=ones_row[0:1, 0:D],
                                     rhs=rcpb[:, b, :], start=True, stop=True)
                og = sb.tile([D, 2, QC], BF16, tag="avs", bufs=1,
                             name=f"og{qc}")
                for b in range(B):
                    nc.vector.tensor_mul(og[:, b, :], av[0:D, b, :],
                                         sgb[b][:, qsl])
                nc.vector.scalar_tensor_tensor(og, og, 1.0, rb[0:D, :, :],
                                               OP.mult, OP.mult)
                half = qc // 2
                dstbuf = a_oA_i if half == 0 else a_oB_i
                for b in range(B):
                    for hh in range(2):
                        (nc.sync if hh else nc.scalar).dma_start(
                            dstbuf[4 * b + 2 * (qc % 2) + hh],
                            og[:, b, 256 * hh:256 * hh + 256])

                # interleave phase-3 work (staged to keep the exp stream hot)
                if qc == 1:
                    a2a(a_oA_i, a_oA_o)
                    load_x3(0, a_oA_o)
                    sts_oA = stats_batch([x3], cols=(0, 256))
                    silus["o"] = sb3.tile([128, NC_IN, R], BF16, tag="silu",
                                          name="silu_o")
                    xn_oA = prep_a(x3, "o", cols=(0, 256), silu_exp=True,
                                   stats=sts_oA[0])
                if qc == 2:
                    st_o = prep_b(xn_oA, "o", want_f8=False, cols=(0, 256))
                if qc == 3:
                    lo_mm_tiles[0] = ps_mm.tile([128, 2, R], F32, tag="mm",
                                                name="mm_lo0")
                    lo_mms(0, (0, 256), lo_mm_tiles[0])
                    a2a(a_oB_i, a_oB_o)
                    load_x3(1, a_oB_o)

            # phase-3 tail
            if stop > 1:
                xn_oB = prep_a(x3, "o", cols=(256, R), silu_exp=True)
                prep_b(xn_oB, "o", want_f8=False, cols=(256, R),
                       reuse=st_o)
                lo_mms(0, (256, R), lo_mm_tiles[0])
                eo = sb.tile([128, 2, R], BF16, tag="eof", bufs=1,
                             name="eo_out0")
                nc.scalar.activation(eo, lo_mm_tiles[0], AF.Identity)
                nc.gpsimd.dma_start(io["outT"][0], eo)
                mm1 = ps_s.tile([128, 2, R], F32, tag="S", name="mm_lo1")
                lo_mms(1, (0, R), mm1)
                eo1 = sb.tile([128, 2, R], BF16, tag="eof", bufs=1,
                              name="eo_out1")
                nc.scalar.activation(eo1, mm1, AF.Identity)
                nc.gpsimd.dma_start(io["outT"][1], eo1)

    nc.compile()
    return nc


# ------------------------------------------------------------------------- host
def _f8(x):
    return np.clip(np.asarray(x, np.float32), -448, 448).astype(F8NP)


def _bf(x):
    return np.asarray(x, np.float32).astype(BFNP)


def _prep_weights(inputs):
    w = {}
    ws = {}
    for l, sc in (("lq", float(D) ** -0.5), ("lk", 1.0), ("lv", 1.0),
                  ("lg", 1.0), ("lo", 1.0)):
        sw = np.asarray(inputs[l + "_sw"], np.float32) * sc
        bw = np.asarray(inputs[l + "_bw"], np.float32) * sc
        assert np.allclose(np.asarray(inputs[l + "_bb"]), 0.0), "bias != 0"
        assert np.all(np.asarray(inputs[l + "_ln_s"]) == 1.0)
        assert np.all(np.asarray(inputs[l + "_ln_b"]) == 0.0)
        if l == "lo":
            swp = _bf(sw.reshape(OUT, NC_IN, 128, G).transpose(3, 1, 2, 0))
            w["lo_swp"] = np.ascontiguousarray(swp)
            w["lo_bwp"] = np.ascontiguousarray(_bf(bw.T.reshape(NC_IN, 128, OUT)))
            ws[l] = 1.0
        else:
            s = 2.0 ** np.floor(np.log2(112.0 / np.abs(sw).max()))
            ws[l] = float(s)
            # sw [out, in*G]; in = c*128+p, c = 2*cp+ko -> [pair=(j,cp),p,ko,out]
            sw_r = (sw * s).reshape(4, 128, 2, 2, 128, G)  # [m,mc,cp,ko,p,j]
            sw8 = sw_r.transpose(0, 4, 5, 2, 3, 1).reshape(4, 128, 16, 2, 128)
            w[l + "_sw8"] = np.ascontiguousarray(_f8(sw8))
            w[l + "_bwp"] = np.ascontiguousarray(
                _bf((bw * s).T.reshape(NC_IN, 128, OUT)))
    return w, ws


def kernel(**inputs):
    w, ws = _prep_weights(inputs)
    key = tuple(sorted(ws.items()))
    if _cache.get("key") != key:
        _cache["nc"] = _build_program(ws)
        _cache["key"] = key
    nc = _cache["nc"]

    q = np.asarray(inputs["q"], np.float32).reshape(B * L, IN)
    k = np.asarray(inputs["k"], np.float32).reshape(B * L, IN)
    v = np.asarray(inputs["v"], np.float32).reshape(B * L, IN)

    in_maps = []
    for core in range(NCORES):
        rows = slice(R * core, R * (core + 1))
        xT3 = np.stack([np.ascontiguousarray(_bf(q[rows].T)),
                        np.ascontiguousarray(_bf(k[rows].T)),
                        np.ascontiguousarray(_bf(v[rows].T))])
        m = {"xT3": xT3}
        m.update(w)
        in_maps.append(m)

    trace = bool(int(os.environ.get("KERNEL_TRACE", "0")))
    res = run_bass_kernel_spmd(nc, in_maps, core_ids=list(range(NCORES)),
                               trace=trace)
    _cache["last_result"] = res

    # unshard: core r holds batch r//4, q ranges [(r%4)*256, +256) and
    # [1024+(r%4)*256, +256); outT [2(m-big), 128, 2(mi), R]
    out = np.zeros((B, L, OUT), np.float32)
    for core in range(NCORES):
        o = res.results[core]["outT"].reshape(2, 128, 2, R)
        o = o.transpose(0, 2, 1, 3).reshape(OUT, R)   # [outdim, rows]
        b = core // 4
        q0 = (core % 4) * 256
        out[b, q0:q0 + 256, :] = o[:, 0:256].T
        out[b, 1024 + q0:1024 + q0 + 256, :] = o[:, 256:R].T
    return out


# revision 56
# speedup vs baseline: 1.0810x; 1.0470x over previous
"""AttentionWithFastKANTransform Trainium2 kernel (8 NeuronCores, SPMD).

v2 design:
  phase 1 (row-sharded, R=512 rows/core): FastKAN lq/lk/lv/lg with fp8
    DoubleRow spline matmuls (4x fewer PE cycles). RBF basis built by a
    bf16 multiply chain on DVE (b_{j+1} = b_j * rc_j, rc_{j+1} = rc_j*e^-2)
    seeded by two ACT exps, then converted to fp8 tiles for the matmuls.
  AllToAlls (fp8): wq/wk -> [32,2ko,L] per head; wv locally PE-transposed
    to [k,d] before the a2a; sigmoid gate bf16.
  phase 2 (head-sharded): S = wk^T wq fp8 DoubleRow (k-partitioned, 32x2
    contraction), exp on ACT -> fp8 A pair-tiles, att@V fp8 DoubleRow over
    k-tile pairs with a ones-column for softmax denominators.
  Gated output a2a'd back in two halves (bf16) so phase 3 overlaps phase 2.
  phase 3: FastKAN lo with bf16 spline (fp8 too lossy for the final layer),
    split in two row-halves for overlap.
"""

import os
import numpy as np
import ml_dtypes

import concourse.bass as bass
import concourse.bacc as bacc
import concourse.tile as tile
import concourse.mybir as mybir
from concourse.bass_utils import run_bass_kernel_spmd
from concourse.masks import make_identity

AF = mybir.ActivationFunctionType
OP = mybir.AluOpType
F32 = mybir.dt.float32
BF16 = mybir.dt.bfloat16
F8 = mybir.dt.float8e4
F8NP = ml_dtypes.float8_e4m3fn
BFNP = ml_dtypes.bfloat16

NCORES = 8
B, L, IN, OUT, H, D, G = 2, 2048, 512, 512, 8, 64, 8
R = (B * L) // NCORES          # 512 rows per core
NC_IN = IN // 128              # 4 input-dim chunks
NKT = L // 128                 # 16 k-tiles per batch
STEP = 4.0 / (G - 1)
EPS = 1e-5
QC = 512                       # phase-2 q-chunk
NQC = L // QC                  # 4
F8_LAYERS = ("lq", "lk", "lv", "lg")
RHO = float(np.exp(-2.0))

_cache = {}


def _patch_act_tables():
    """Prefer the ln+exp combined table so Ln/Exp don't ping-pong loads."""
    if _cache.get("act_patched"):
        return
    import concourse.bacc as _bacc
    import concourse.hw_specs as _hw
    orig = _hw.get_activation_tables

    def reordered(arch):
        t = dict(orig(arch))
        order = ["natural_log_exp_and_others", "silu_and_others"]
        out = {k: t[k] for k in order if k in t}
        out.update({k: v for k, v in t.items() if k not in out})
        return out

    _bacc.get_activation_tables = reordered
    _cache["act_patched"] = True


def _build_program(ws):
    """ws: dict layer -> fp8 weight scale (host-derived, baked as consts)."""
    _patch_act_tables()
    nc = bacc.Bacc("TRN2", target_bir_lowering=False, debug=False,
                   num_devices=NCORES)
    io = {}
    io["xT3"] = nc.dram_tensor("xT3", [3, IN, R], BF16, kind="ExternalInput").ap()
    for l in F8_LAYERS:
        io[l + "_sw8"] = nc.dram_tensor(l + "_sw8", [4, 128, 16, 2, 128], F8,
                                        kind="ExternalInput").ap()
    io["lo_swp"] = nc.dram_tensor("lo_swp", [G, NC_IN, 128, OUT], BF16,
                                  kind="ExternalInput").ap()
    for l in F8_LAYERS + ("lo",):
        io[l + "_bwp"] = nc.dram_tensor(l + "_bwp", [NC_IN, 128, OUT], BF16,
                                        kind="ExternalInput").ap()
    io["outT"] = nc.dram_tensor("outT", [2, 128, 2, R], BF16,
                                kind="ExternalOutput").ap()

    rg = [list(range(NCORES))]
    nocc = bool(int(os.environ.get("KERNEL_NOCC", "0")))
    stop = int(os.environ.get("KERNEL_STOP", "9"))

    with tile.TileContext(nc) as tc:
        with tc.tile_pool(name="dram1", bufs=1, space="DRAM") as dram1, \
             tc.tile_pool(name="sb", bufs=2) as sb, \
             tc.tile_pool(name="sb3", bufs=3) as sb3, \
             tc.tile_pool(name="ub", bufs=3) as ubp, \
             tc.tile_pool(name="ubo", bufs=8) as ubop, \
             tc.tile_pool(name="wt", bufs=2) as wtp, \
             tc.tile_pool(name="consts", bufs=1) as cpool, \
             tc.tile_pool(name="ps_mm", bufs=2, space="PSUM") as ps_mm, \
             tc.tile_pool(name="ps_s", bufs=2, space="PSUM") as ps_s:

            # ---------------- collective buffers
            a_qk_i = dram1.tile([NCORES, 2, D, R], F8, tag="aqk_i")
            a_qk_o = dram1.tile([NCORES, 2, D, R], F8, tag="aqk_o")
            a_sg_i = dram1.tile([NCORES, D, R], BF16, tag="asg_i")
            a_sg_o = dram1.tile([NCORES, D, R], BF16, tag="asg_o")
            a_wv_i = dram1.tile([NCORES, R, D], F8, tag="awv_i")
            a_wv_o = dram1.tile([NCORES, R, D], F8, tag="awv_o")
            a_oA_i = dram1.tile([NCORES, D, R // 2], BF16, tag="aoA_i")
            a_oA_o = dram1.tile([NCORES, D, R // 2], BF16, tag="aoA_o")
            a_oB_i = dram1.tile([NCORES, D, R // 2], BF16, tag="aoB_i")
            a_oB_o = dram1.tile([NCORES, D, R // 2], BF16, tag="aoB_o")

            def a2a(i, o):
                if nocc:
                    nc.sync.dma_start(o, i)
                else:
                    nc.gpsimd.collective_compute(
                        "AllToAll", OP.bypass, replica_groups=rg,
                        ins=[i.opt()], outs=[o.opt()])

            # ---------------- consts
            ones_col = cpool.tile([128, 1], BF16, tag="ones_col")
            nc.vector.memset(ones_col, 1.0 / IN)
            ones_row = cpool.tile([128, 128], BF16, tag="ones_row")
            nc.vector.memset(ones_row, 1.0)
            ones_rowf = cpool.tile([128, 128], F32, tag="ones_rowf")
            nc.vector.memset(ones_rowf, 1.0)
            ident8 = cpool.tile([128, 128], F8, tag="ident8")
            make_identity(nc, ident8)
            rho1 = cpool.tile([128, 1, R], BF16, tag="rho")
            nc.vector.memset(rho1, RHO)
            bm25 = cpool.tile([128, 1], F32, tag="bm25")
            nc.vector.memset(bm25, 3.5 - 6.0)
            bm35 = cpool.tile([128, 1], F32, tag="bm35")
            nc.vector.memset(bm35, 3.5 - 7.0)
            bm05 = cpool.tile([128, 1], F32, tag="bm05")
            nc.vector.memset(bm05, 3.5 - 4.0)
            bm20 = cpool.tile([128, 1], F32, tag="bm20")
            nc.vector.memset(bm20, 6.0 - 8.0)
            epst = cpool.tile([128, 1], F32, tag="eps")
            nc.vector.memset(epst, EPS)
            b35 = cpool.tile([128, 1], F32, tag="b35")
            nc.vector.memset(b35, 3.5)
            b60 = cpool.tile([128, 1], F32, tag="b60")
            nc.vector.memset(b60, 6.0)

            # PE warmup: keep the array busy during input DMA so the
            # first real matmuls run at full clock
            warm_ps = ps_s.tile([128, 2, R], F32, tag="S", name="warm")
            for w in range(10):
                nc.tensor.matmul(warm_ps[:, 0, 0:128], lhsT=ones_row,
                                 rhs=ones_row, start=True, stop=True,
                                 skip_group_check=True)

            # ---------------- x loads + batched silus (one table switch)
            def load_x(idx):
                x = sb3.tile([128, NC_IN, R], BF16, tag="x", name=f"x{idx}")
                nc.sync.dma_start(
                    x, io["xT3"][idx].rearrange("(c p) r -> p c r", p=128))
                return x

            xk, xq, xv = load_x(1), load_x(0), load_x(2)
            silus = {}
            for nm, x in (("k", xk), ("q", xq), ("v", xv)):
                s = sb3.tile([128, NC_IN, R], BF16, tag="silu", name=f"silu_{nm}")
                nc.scalar.activation(s, x, AF.Silu)
                silus[nm] = s

            # ---------------- batched LN stats (k,q,v in one Ln/Exp pair)
            def stats_batch(xs, cols=None):
                """xs: list of (x_sb, xsq_writer) tensors; returns list of
                (s_ap, t_ap) [1, n] access patterns per tensor."""
                lo_, hi_ = cols or (0, R)
                n = hi_ - lo_
                csl = slice(lo_, hi_)
                nt = len(xs)
                stat = ps_s.tile([97, 2, R], F32, tag="S", name="statb")
                for t, x_sb in enumerate(xs):
                    xsq = ubp.tile([128, NC_IN, R], BF16, tag="u",
                                   name=f"xsqb{t}")
                    for c in range(NC_IN):
                        nc.tensor.matmul(stat[32 * t:32 * t + 1, 0, csl],
                                         lhsT=ones_col, rhs=x_sb[:, c, csl],
                                         start=(c == 0), stop=(c == NC_IN - 1))
                    nc.vector.tensor_mul(xsq[:, :, csl], x_sb[:, :, csl],
                                         x_sb[:, :, csl])
                    for c in range(NC_IN):
                        nc.tensor.matmul(stat[32 * t:32 * t + 1, 1, csl],
                                         lhsT=ones_col, rhs=xsq[:, c, csl],
                                         start=(c == 0), stop=(c == NC_IN - 1))
                # stat rows now hold mu and E[x^2] directly (1/IN folded)
                sm = sb.tile([97, 3, R], F32, tag="stsm", bufs=1, name="smb")
                nc.gpsimd.memset(sm, 1.0)
                for t in range(nt):
                    p = slice(32 * t, 32 * t + 1)
                    var = sm[p, 1, csl]
                    # mumu = mu^2 (one PSUM operand is allowed)
                    nc.vector.scalar_tensor_tensor(sm[p, 2, csl],
                                                   stat[p, 0, csl], 1.0,
                                                   stat[p, 0, csl],
                                                   OP.mult, OP.mult)
                    nc.vector.tensor_sub(var, stat[p, 1, csl],
                                         sm[p, 2, csl])
                # one Ln + one Exp over all tensors (spread on partitions)
                nc.scalar.activation(sm[:, 2, csl], sm[:, 1, csl],
                                     AF.Ln, bias=epst[0:97])
                nc.scalar.activation(sm[:, 1, csl], sm[:, 2, csl],
                                     AF.Exp, scale=-0.5)
                for t in range(nt):
                    p = slice(32 * t, 32 * t + 1)
                    nc.vector.scalar_tensor_tensor(sm[p, 2, csl],
                                                   stat[p, 0, csl], -1.0,
                                                   sm[p, 1, csl],
                                                   OP.mult, OP.mult)
                return [(sm[32 * t:32 * t + 1, 1, :],
                         sm[32 * t:32 * t + 1, 2, :]) for t in range(nt)]

            # ---------------- prep stage A: xn (+ phase-3 silu)
            def prep_a(x_sb, nm, cols=None, silu_exp=False, stats=None):
                lo_, hi_ = cols or (0, R)
                n = hi_ - lo_
                csl = slice(lo_, hi_)

                if stats is None:
                    stats = stats_batch([x_sb], cols=cols)[0]
                s_ap, t_ap = stats
                # broadcast via PE: [1,n] -> [128,n] (two mms, one per bank)
                bp = s_ap.base_partition()
                orow = ones_rowf[bp:bp + 1, :].bitcast(mybir.dt.float32r)
                stb_ps = ps_s.tile([128, 2, R], F32, tag="S", name=f"stb_{nm}")
                nc.tensor.matmul(stb_ps[:, 0, csl], lhsT=orow,
                                 rhs=s_ap[:, csl].bitcast(mybir.dt.float32r),
                                 start=True, stop=True)
                nc.tensor.matmul(stb_ps[:, 1, csl], lhsT=orow,
                                 rhs=t_ap[:, csl].bitcast(mybir.dt.float32r),
                                 start=True, stop=True)
                st_bc = sb.tile([128, 2, R], BF16, tag="stbc", bufs=1, name=f"stbc_{nm}")
                nc.vector.tensor_copy(st_bc[:, :, csl], stb_ps[:, :, csl])

                xn = sb.tile([128, NC_IN, R], BF16, tag="xn", bufs=3, name=f"xn_{nm}")
                for c in range(NC_IN):
                    nc.vector.tensor_mul(xn[:, c, csl], x_sb[:, c, csl],
                                         st_bc[:, 0, csl])
                    nc.vector.tensor_add(xn[:, c, csl], xn[:, c, csl],
                                         st_bc[:, 1, csl])

                # silu via exp route (phase 3; avoids a table switch)
                if silu_exp:
                    e = ubp.tile([128, NC_IN, R], BF16, tag="u", name=f"se_{nm}")
                    nc.scalar.activation(e[:, :, csl], x_sb[:, :, csl],
                                         AF.Exp, scale=-1.0)
                    with nc.allow_low_precision(reason="sigmoid gate bf16"):
                        nc.vector.tensor_scalar(e[:, :, csl], e[:, :, csl],
                                                1.0, None, OP.add)
                        nc.vector.reciprocal(e[:, :, csl], e[:, :, csl])
                    so = silus[nm]
                    nc.vector.tensor_mul(so[:, :, csl], x_sb[:, :, csl],
                                         e[:, :, csl])
                return xn

            # ---------------- prep stage B: seeds + chain + f8 conversion
            def prep_b(xn, nm, want_f8=True, cols=None, reuse=None):
                lo_, hi_ = cols or (0, R)
                n = hi_ - lo_
                csl = slice(lo_, hi_)
                # seeds: zsq = Square(1.75*xn + 3.5); b0 = Exp(-zsq);
                # rc0 = Exp(3.5*xn + 6).  zsq scratch lives in PSUM (bitcast).
                def psum_bf16(name):
                    t = ps_s.tile([128, 2, R], F32, tag="S", name=name)
                    return t.bitcast(BF16).rearrange(
                        "p a (b r) -> p (a b) r", b=2)
                zsq = psum_bf16(f"zq_{nm}")
                nc.scalar.activation(zsq[:, :, csl], xn[:, :, csl], AF.Square,
                                     scale=1.0 / STEP, bias=b35)
                def new_u(j):
                    if reuse is not None:
                        return reuse["us"][j]
                    if want_f8:
                        return ubp.tile([128, NC_IN, R], BF16, tag="u",
                                        name=f"u{j}_{nm}")
                    return ubop.tile([128, NC_IN, R], BF16, tag="ub8",
                                     name=f"u{j}_{nm}")

                zsq4 = psum_bf16(f"zq4_{nm}")
                nc.scalar.activation(zsq4[:, :, csl], xn[:, :, csl], AF.Square,
                                     scale=1.0 / STEP, bias=bm05)
                rc_prev = sb.tile([128, NC_IN, R], BF16, tag="rc", bufs=3,
                                  name=f"rc0_{nm}")
                nc.scalar.activation(rc_prev[:, :, csl], xn[:, :, csl],
                                     AF.Exp, scale=2.0 / STEP, bias=b60)
                rc4 = sb.tile([128, NC_IN, R], BF16, tag="rc", bufs=3,
                              name=f"rc4_{nm}")
                nc.scalar.activation(rc4[:, :, csl], xn[:, :, csl],
                                     AF.Exp, scale=2.0 / STEP, bias=bm20)
                us = [new_u(0)]
                nc.scalar.activation(us[0][:, :, csl], zsq[:, :, csl],
                                     AF.Exp, scale=-1.0)

                basis8 = None
                if want_f8:
                    basis8 = sb.tile([128, G, 2, 2, R], F8, tag="b8",
                                     bufs=3, name=f"b8_{nm}")

                def conv(u_t, j):
                    if not want_f8:
                        return
                    dst = basis8[:, j, :, :, csl]
                    src = u_t[:, :, csl].rearrange("p (cp ko) r -> p cp ko r",
                                                   cp=2)
                    if j in (0, 1, 2, 3):
                        nc.gpsimd.tensor_copy(dst, src)
                    else:
                        nc.vector.tensor_copy(dst, src)

                conv(us[0], 0)
                # chain A: j = 1..3 from u0
                for j in range(1, 4):
                    us.append(new_u(j))
                    nc.vector.tensor_mul(us[j][:, :, csl],
                                         us[j - 1][:, :, csl],
                                         rc_prev[:, :, csl])
                    conv(us[j], j)
                    if j < 3:
                        rc_t = sb.tile([128, NC_IN, R], BF16, tag="rc", bufs=3,
                                       name=f"rc{j}_{nm}")
                        nc.vector.tensor_mul(rc_t[:, :, csl],
                                             rc_prev[:, :, csl],
                                             rho1[:, :, csl].to_broadcast(
                                                 (128, NC_IN, n)))
                        rc_prev = rc_t
                # chain B: seed u4 (from zsq4), then j = 5 (+6,7 for bf16)
                us.append(new_u(4))
                nc.scalar.activation(us[4][:, :, csl], zsq4[:, :, csl],
                                     AF.Exp, scale=-1.0)
                conv(us[4], 4)
                jend = 7
                for j in range(5, jend + 1):
                    us.append(new_u(j))
                    nc.vector.tensor_mul(us[j][:, :, csl],
                                         us[j - 1][:, :, csl],
                                         rc4[:, :, csl])
                    conv(us[j], j)
                    if j < jend:
                        rc_t = sb.tile([128, NC_IN, R], BF16, tag="rc", bufs=3,
                                       name=f"rc4{j}_{nm}")
                        nc.vector.tensor_mul(rc_t[:, :, csl],
                                             rc4[:, :, csl],
                                             rho1[:, :, csl].to_broadcast(
                                                 (128, NC_IN, n)))
                        rc4 = rc_t
                return {"b8": basis8, "us": us}

            # ---------------- fp8 layer matmuls + epilogues
            DMA_ENGS = (nc.sync, nc.scalar, nc.gpsimd)

            def mm_f8(lname, st, silu, epi):
                for mt in range(2):
                    mm = ps_mm.tile([128, 2, R], F32, tag="mm",
                                    name=f"mm_{lname}{mt}")
                    for mi in range(2):
                        m = 2 * mt + mi
                        wt8 = wtp.tile([128, 16, 2, 128], F8, tag="wt8")
                        nc.sync.dma_start(wt8, io[lname + "_sw8"][m])
                        bwt = wtp.tile([128, NC_IN, 128], BF16, tag="bwt")
                        nc.sync.dma_start(
                            bwt, io[lname + "_bwp"][:, :,
                                                    128 * m:128 * (m + 1)]
                            .rearrange("c p m -> p c m"))
                        for pair in range(16):
                            nc.tensor.matmul(
                                mm[:, mi, :], lhsT=wt8[:, pair, :, :],
                                rhs=st["b8"][:, pair // 2, pair % 2, :, :],
                                start=(pair == 0), stop=False,
                                perf_mode=mybir.MatmulPerfMode.DoubleRow)
                        for c in range(NC_IN):
                            nc.tensor.matmul(
                                mm[:, mi, :], lhsT=bwt[:, c, :],
                                rhs=silu[:, c, :],
                                start=False, stop=(c == NC_IN - 1))
                    epi(mm, mt)

            def epi_qk(ttype, scale):
                def _e(mm, mt):
                    eo = sb.tile([128, 2, R], F8, tag="eo8",
                                 name=f"eoqk{ttype}{mt}")
                    nc.scalar.activation(eo, mm, AF.Identity, scale=scale)
                    for mi in range(2):
                        nc.scalar.dma_start(
                            a_qk_i[4 * mt + 2 * mi:4 * mt + 2 * mi + 2, ttype],
                            eo[:, mi, :].rearrange("(h2 d) r -> h2 d r", h2=2))
                return _e

            def epi_sg(scale):
                def _e(mm, mt):
                    e = sb.tile([128, 2, R], BF16, tag="eob", bufs=1, name=f"eosg{mt}")
                    nc.scalar.activation(e, mm, AF.Exp, scale=-scale)
                    with nc.allow_low_precision(reason="sigmoid gate bf16"):
                        nc.gpsimd.tensor_scalar(e, e, 1.0, None, OP.add)
                        nc.vector.reciprocal(e, e)
                    for mi in range(2):
                        nc.scalar.dma_start(
                            a_sg_i[4 * mt + 2 * mi:4 * mt + 2 * mi + 2],
                            e[:, mi, :].rearrange("(h2 d) r -> h2 d r", h2=2))
                return _e

            def epi_wv(scale):
                def _e(mm, mt):
                    eo = sb.tile([128, 2, R], F8, tag="eo8", name=f"eowv{mt}")
                    nc.scalar.activation(eo, mm, AF.Identity, scale=scale)
                    # transpose [64,128] blocks -> [rows, d] and ship
                    for mi in range(2):
                        for h2 in range(2):
                            tp = ps_mm.tile([128, 2, R], F32, tag="mm",
                                            name=f"tp{mt}{mi}{h2}")
                            tp8 = tp[:, 0, 0:64].bitcast(F8)
                            tpv = tp8.rearrange("p (rc d) -> p rc d", rc=4)
                            for rc in range(4):
                                nc.tensor.transpose(
                                    tpv[:, rc, :],
                                    eo[64 * h2:64 * h2 + 64, mi,
                                       128 * rc:128 * rc + 128],
                                    ident8[64 * h2:64 * h2 + 64,
                                           64 * h2:64 * h2 + 64])
                            stg = sb.tile([128, 4, D], F8, tag="wvstg",
                                          name=f"wvstg{mt}{mi}{h2}")
                            nc.vector.tensor_copy(stg, tpv)
                            nc.scalar.dma_start(
                                a_wv_i[2 * (2 * mt + mi) + h2]
                                .rearrange("(rc p) d -> p rc d", rc=4),
                                stg)
                return _e

            # ---------------- phase 1 schedule
            sts = stats_batch([xk, xq, xv])
            xn_k = prep_a(xk, "k", stats=sts[0])
            xn_q = prep_a(xq, "q", stats=sts[1])
            xn_v = prep_a(xv, "v", stats=sts[2])
            st_k = prep_b(xn_k, "k")
            st_q = prep_b(xn_q, "q")
            st_v = prep_b(xn_v, "v")
            mm_f8("lk", st_k, silus["k"], epi_qk(1, 1.0 / ws["lk"]))
            mm_f8("lq", st_q, silus["q"], epi_qk(0, 1.0 / ws["lq"]))
            a2a(a_qk_i, a_qk_o)
            mm_f8("lv", st_v, silus["v"], epi_wv(1.0 / ws["lv"]))
            a2a(a_wv_i, a_wv_o)
            mm_f8("lg", st_q, silus["q"], epi_sg(1.0 / ws["lg"]))
            a2a(a_sg_i, a_sg_o)

            # ---------------- phase 2 receive tiles
            wqb, wkb, wva, sgb = [], [], [], []
            wq_pk = wk_pk = None
            if stop > 1:
                wq_pk = sb.tile([64, 2, L], F8, tag="wqpk", bufs=1)
                wk_pk = sb.tile([64, 2, L], F8, tag="wkpk", bufs=1)
                engs = (nc.sync, nc.scalar, nc.gpsimd, nc.sync)
                for b in range(B):
                    for t, ty in ((wq_pk, 0), (wk_pk, 1)):
                        for s in range(4):
                            engs[s].dma_start(
                                t[32 * b:32 * b + 32, :,
                                  512 * s:512 * (s + 1)],
                                a_qk_o[4 * b + s, ty]
                                .rearrange("(ko ki) r -> ki ko r", ko=2))
                    wqb.append(wq_pk[32 * b:32 * b + 32, :, :])
                    wkb.append(wk_pk[32 * b:32 * b + 32, :, :])
            for b in range(B) if stop > 1 else []:
                t = sb.tile([128, 8, 2, D + 1], F8, tag=f"wva{b}", bufs=1)
                nc.vector.memset(t[:, :, :, D:D + 1], 1.0)
                for s in range(4):
                    (nc.gpsimd if s % 2 else nc.scalar).dma_start(
                        t[:, 2 * s:2 * s + 2, :, 0:D],
                        a_wv_o[4 * b + s].rearrange(
                            "(pr par p) d -> p pr par d", pr=2, par=2))
                wva.append(t)
                t = sb.tile([D, L], BF16, tag=f"sgb{b}", bufs=1)
                for s in range(4):
                    (nc.scalar if s % 2 else nc.sync).dma_start(
                        t[:, 512 * s:512 * (s + 1)], a_sg_o[4 * b + s])
                sgb.append(t)

            # ---------------- phase 2/3 interleaved
            x3 = sb.tile([128, NC_IN, R], BF16, tag="x", name="x3")
            st_o = None

            def load_x3(hq, src):
                engs = (nc.sync, nc.scalar, nc.gpsimd, nc.sync)
                for c in range(NC_IN):
                    for h2 in range(2):
                        engs[c].dma_start(
                            x3[64 * h2:64 * h2 + 64, c,
                               256 * hq:256 * hq + 256],
                            src[2 * c + h2])

            def lo_mms(mt, cols, mm):
                lo_, hi_ = cols
                csl = slice(lo_, hi_)
                for mi in range(2):
                    m = 2 * mt + mi
                    bwt = wtp.tile([128, NC_IN, 128], BF16, tag="bwt")
                    nc.sync.dma_start(
                        bwt, io["lo_bwp"][:, :, 128 * m:128 * (m + 1)]
                        .rearrange("c p m -> p c m"))
                    for kh in range(4):
                        wt = wtp.tile([128, 8, 128], BF16, tag="wtlo",
                                      bufs=2)
                        DMA_ENGS[kh % 3].dma_start(
                            wt, io["lo_swp"][2 * kh:2 * kh + 2, :, :,
                                             128 * m:128 * (m + 1)]
                            .rearrange("j c i m -> i (j c) m"))
                        for kk in range(8):
                            kc = 8 * kh + kk
                            nc.tensor.matmul(
                                mm[:, mi, csl], lhsT=wt[:, kk, :],
                                rhs=st_o["us"][kc // NC_IN][:, kc % NC_IN, csl],
                                start=(kc == 0), stop=False)
                    for c in range(NC_IN):
                        nc.tensor.matmul(mm[:, mi, csl],
                                         lhsT=bwt[:, c, :],
                                         rhs=silus["o"][:, c, csl],
                                         start=False, stop=(c == NC_IN - 1))

            lo_mm_tiles = {}

            for qc in range(NQC) if stop > 1 else []:
                qsl = slice(QC * qc, QC * (qc + 1))
                av_t = ps_mm.tile([128, 2, QC], F32, tag="mm",
                                  name=f"av{qc}")
                av = av_t[0:D + 1, :, :]
                a8_t = None
                for kt in range(NKT):
                    S = ps_s.tile([128, 2, QC], F32, tag="S", name=f"S{qc}_{kt}")
                    for b in range(B):
                        nc.tensor.matmul(
                            S[:, b, :],
                            lhsT=wkb[b][:, :, 128 * kt:128 * (kt + 1)],
                            rhs=wqb[b][:, :, qsl],
                            start=True, stop=True,
                            perf_mode=mybir.MatmulPerfMode.DoubleRow)
                    if kt % 2 == 0:
                        a8_t = sb.tile([128, 2, 2, QC], F8, tag="a8",
                                       name=f"a8_{qc}_{kt // 2}")
                    nc.scalar.activation(a8_t[:, kt % 2, :, :], S, AF.Exp)
                    if kt % 2 == 1:
                        for b in range(B):
                            nc.tensor.matmul(
                                av[:, b, :],
                                lhsT=wva[b][:, kt // 2, :, :],
                                rhs=a8_t[:, :, b, :],
                                start=(kt == 1), stop=(kt == NKT - 1),
                                perf_mode=mybir.MatmulPerfMode.DoubleRow)
                # gating: og = av[0:D] * (1/den) * sg
                rcpb = sb.tile([1, 2, QC], BF16, tag="rcpb", bufs=1,
                               name=f"rcpb{qc}")
                with nc.allow_low_precision(reason="softmax denom bf16"):
                    nc.vector.reciprocal(rcpb, av[D:D + 1, :, :])
                rb = ps_s.tile([128, 2, QC], F32, tag="S", name=f"rb{qc}")
                for b in range(B):
                    nc.tensor.matmul(rb[0:D, b, :],
                                     lhsT=ones_row[0:1, 0:D],
                                     rhs=rcpb[:, b, :], start=True, stop=True)
                og = sb.tile([D, 2, QC], BF16, tag="avs", bufs=1,
                             name=f"og{qc}")
                for b in range(B):
                    nc.vector.tensor_mul(og[:, b, :], av[0:D, b, :],
                                         sgb[b][:, qsl])
                nc.vector.scalar_tensor_tensor(og, og, 1.0, rb[0:D, :, :],
                                               OP.mult, OP.mult)
                half = qc // 2
                dstbuf = a_oA_i if half == 0 else a_oB_i
                for b in range(B):
                    for hh in range(2):
                        (nc.sync if hh else nc.scalar).dma_start(
                            dstbuf[4 * b + 2 * (qc % 2) + hh],
                            og[:, b, 256 * hh:256 * hh + 256])

                # interleave phase-3 work (staged to keep the exp stream hot)
                if qc == 1:
                    a2a(a_oA_i, a_oA_o)
                    load_x3(0, a_oA_o)
                    sts_oA = stats_batch([x3], cols=(0, 256))
                    silus["o"] = sb3.tile([128, NC_IN, R], BF16, tag="silu",
                                          name="silu_o")
                    xn_oA = prep_a(x3, "o", cols=(0, 256), silu_exp=True,
                                   stats=sts_oA[0])
                if qc == 2:
                    st_o = prep_b(xn_oA, "o", want_f8=False, cols=(0, 256))
                if qc == 3:
                    lo_mm_tiles[0] = ps_mm.tile([128, 2, R], F32, tag="mm",
                                                name="mm_lo0")
                    lo_mms(0, (0, 256), lo_mm_tiles[0])
                    a2a(a_oB_i, a_oB_o)
                    load_x3(1, a_oB_o)

            # phase-3 tail
            if stop > 1:
                xn_oB = prep_a(x3, "o", cols=(256, R), silu_exp=True)
                prep_b(xn_oB, "o", want_f8=False, cols=(256, R),
                       reuse=st_o)
                lo_mms(0, (256, R), lo_mm_tiles[0])
                eo = sb.tile([128, 2, R], BF16, tag="eof", bufs=1,
                             name="eo_out0")
                nc.scalar.activation(eo, lo_mm_tiles[0], AF.Identity)
                nc.gpsimd.dma_start(io["outT"][0], eo)
                mm1 = ps_s.tile([128, 2, R], F32, tag="S", name="mm_lo1")
                lo_mms(1, (0, R), mm1)
                eo1 = sb.tile([128, 2, R], BF16, tag="eof", bufs=1,
                              name="eo_out1")
                nc.scalar.activation(eo1, mm1, AF.Identity)
                nc.gpsimd.dma_start(io["outT"][1], eo1)

    nc.compile()
    return nc


# ------------------------------------------------------------------------- host
def _f8(x):
    return np.clip(np.asarray(x, np.float32), -448, 448).astype(F8NP)


def _bf(x):
    return np.asarray(x, np.float32).astype(BFNP)


def _prep_weights(inputs):
    w = {}
    ws = {}
    for l, sc in (("lq", float(D) ** -0.5), ("lk", 1.0), ("lv", 1.0),
                  ("lg", 1.0), ("lo", 1.0)):
        sw = np.asarray(inputs[l + "_sw"], np.float32) * sc
        bw = np.asarray(inputs[l + "_bw"], np.float32) * sc
        assert np.allclose(np.asarray(inputs[l + "_bb"]), 0.0), "bias != 0"
        assert np.all(np.asarray(inputs[l + "_ln_s"]) == 1.0)
        assert np.all(np.asarray(inputs[l + "_ln_b"]) == 0.0)
        if l == "lo":
            swp = _bf(sw.reshape(OUT, NC_IN, 128, G).transpose(3, 1, 2, 0))
            w["lo_swp"] = np.ascontiguousarray(swp)
            w["lo_bwp"] = np.ascontiguousarray(_bf(bw.T.reshape(NC_IN, 128, OUT)))
            ws[l] = 1.0
        else:
            s = 2.0 ** np.floor(np.log2(112.0 / np.abs(sw).max()))
            ws[l] = float(s)
            # sw [out, in*G]; in = c*128+p, c = 2*cp+ko -> [pair=(j,cp),p,ko,out]
            sw_r = (sw * s).reshape(4, 128, 2, 2, 128, G)  # [m,mc,cp,ko,p,j]
            sw8 = sw_r.transpose(0, 4, 5, 2, 3, 1).reshape(4, 128, 16, 2, 128)
            w[l + "_sw8"] = np.ascontiguousarray(_f8(sw8))
            w[l + "_bwp"] = np.ascontiguousarray(
                _bf((bw * s).T.reshape(NC_IN, 128, OUT)))
    return w, ws


def kernel(**inputs):
    w, ws = _prep_weights(inputs)
    key = tuple(sorted(ws.items()))
    if _cache.get("key") != key:
        _cache["nc"] = _build_program(ws)
        _cache["key"] = key
    nc = _cache["nc"]

    q = np.asarray(inputs["q"], np.float32).reshape(B * L, IN)
    k = np.asarray(inputs["k"], np.float32).reshape(B * L, IN)
    v = np.asarray(inputs["v"], np.float32).reshape(B * L, IN)

    in_maps = []
    for core in range(NCORES):
        rows = slice(R * core, R * (core + 1))
        xT3 = np.stack([np.ascontiguousarray(_bf(q[rows].T)),
                        np.ascontiguousarray(_bf(k[rows].T)),
                        np.ascontiguousarray(_bf(v[rows].T))])
        m = {"xT3": xT3}
        m.update(w)
        in_maps.append(m)

    trace = bool(int(os.environ.get("KERNEL_TRACE", "0")))
    res = run_bass_kernel_spmd(nc, in_maps, core_ids=list(range(NCORES)),
                               trace=trace)
    _cache["last_result"] = res

    # unshard: core r holds batch r//4, q ranges [(r%4)*256, +256) and
    # [1024+(r%4)*256, +256); outT [2(m-big), 128, 2(mi), R]
    out = np.zeros((B, L, OUT), np.float32)
    for core in range(NCORES):
        o = res.results[core]["outT"].reshape(2, 128, 2, R)
        o = o.transpose(0, 2, 1, 3).reshape(OUT, R)   # [outdim, rows]
        b = core // 4
        q0 = (core % 4) * 256
        out[b, q0:q0 + 256, :] = o[:, 0:256].T
        out[b, 1024 + q0:1024 + q0 + 256, :] = o[:, 256:R].T
    return out
